# revision 1
# baseline (speedup 1.0000x reference)
"""MCSPN Trainium2 kernel: guidance convs + softmax gates + 4-step CSPN recurrence.

Data-parallel over batch: 8 images -> 8 NeuronCores, one image per core.
Per core:
  phase A: conv3x3 (fp32r matmuls, 18 accum MMs/row) -> bias+ReLU (ACT)
           -> conv1x1 (fp32r) -> exp (ACT) -> per-row DMA scatter into
           gate layout e_all [H=128 part, 76*256 free]
  softmax: 3 adds + reciprocal + 4 muls over [128, 19*256] strided views
  phase B: 4 recurrence steps; left/right via guarded 258-wide windows of h,
           up/down via PE shift-matmuls (sub/super-diagonal fp32r matrices)
           into PSUM; gated sums on DVE + GPSIMD.
"""
import os
import sys

sys.path.insert(0, "/opt/trn_rl_repo")

import numpy as np

B, CIN, H, W = 8, 256, 128, 256
K = 19
MID = 128
KD = 4 * K  # 76
EPS = 1e-5
T_STEPS = 4
WP = W + 2  # guarded row width (258)
RG = 8      # feats rows per DMA chunk


def _build():
    import concourse.bacc as bacc
    import concourse.mybir as mybir
    import concourse.tile as tile
    from concourse import bass

    f32 = mybir.dt.float32
    f32r = mybir.dt.float32r
    Act = mybir.ActivationFunctionType
    Alu = mybir.AluOpType

    nc = bacc.Bacc("TRN2", target_bir_lowering=False)

    feats_d = nc.dram_tensor("feats", [CIN, H, W], f32, kind="ExternalInput")
    logits_d = nc.dram_tensor("logits", [K, H, W], f32, kind="ExternalInput")
    w1t_d = nc.dram_tensor("w1t", [128, 2, 9, MID], f32, kind="ExternalInput")
    bmid_d = nc.dram_tensor("bmid", [MID, 1], f32, kind="ExternalInput")
    w2t_d = nc.dram_tensor("w2t", [MID, KD], f32, kind="ExternalInput")
    b2_d = nc.dram_tensor("b2", [KD, 1], f32, kind="ExternalInput")
    sup_d = nc.dram_tensor("sup", [128, 128], f32, kind="ExternalInput")
    sdn_d = nc.dram_tensor("sdn", [128, 128], f32, kind="ExternalInput")
    out_d = nc.dram_tensor("out", [K, H, W], f32, kind="ExternalOutput")

    with tile.TileContext(nc) as tc:
        # ---- long-lived tensors ----
        with tc.tile_pool(name="persist", bufs=1) as pp, \
             tc.tile_pool(name="hpool", bufs=1) as hp:
            e_all = pp.tile([128, KD * W], f32)           # 76 KB/part
            h_a = hp.tile([128, K * WP], f32r)            # 19.6 KB/part
            h_b = hp.tile([128, K * WP], f32r)
            w2_r = pp.tile([MID, KD], f32r)
            bmid = pp.tile([MID, 1], f32)
            b2c = pp.tile([KD, 1], f32)
            s_up = pp.tile([128, 128], f32r)
            s_dn = pp.tile([128, 128], f32r)
            z32 = pp.tile([128, 64], f32)  # zeros source for f32r guard writes

            nc.vector.memset(z32[:], 0.0)
            nc.sync.dma_start(out=bmid[:], in_=bmid_d[:])
            nc.sync.dma_start(out=b2c[:], in_=b2_d[:])
            with tc.tile_pool(name="stage", bufs=1) as stp:
                w2_f = stp.tile([MID, KD], f32)
                s_up_f = stp.tile([128, 128], f32)
                s_dn_f = stp.tile([128, 128], f32)
                nc.sync.dma_start(out=w2_f[:], in_=w2t_d[:])
                nc.vector.tensor_copy(out=w2_r[:], in_=w2_f[:])
                nc.sync.dma_start(out=s_up_f[:], in_=sup_d[:])
                nc.vector.tensor_copy(out=s_up[:], in_=s_up_f[:])
                nc.sync.dma_start(out=s_dn_f[:], in_=sdn_d[:])
                nc.vector.tensor_copy(out=s_dn[:], in_=s_dn_f[:])

            # ================= phase A: guidance =================
            with tc.tile_pool(name="w1p", bufs=1) as w1p:
                w1_f = w1p.tile([128, 2, 9, MID], f32)
                w1_r = w1p.tile([128, 2, 9, MID], f32r)
                nc.sync.dma_start(out=w1_f[:], in_=w1t_d[:])
                nc.vector.tensor_copy(out=w1_r[:], in_=w1_f[:])

                with tc.tile_pool(name="frows", bufs=3) as frp, \
                     tc.tile_pool(name="xrow", bufs=3) as xrp, \
                     tc.tile_pool(name="estrip", bufs=3) as esp, \
                     tc.tile_pool(name="psA", bufs=3, space="PSUM") as psA, \
                     tc.tile_pool(name="psG", bufs=3, space="PSUM") as psG:
                    n_groups = H // RG
                    ftiles = []  # group idx -> tile [128, 2, RG, WP]
                    for gi in range(n_groups):
                        ft = frp.tile([128, 2, RG, WP], f32r, name=f"ft{gi}",
                                      tag="ft")
                        # zero guard columns (both chunks, all rows) via
                        # rounding copy (memset can't write f32r)
                        nc.vector.tensor_copy(
                            out=ft[:, :, :, 0:WP:WP - 1],
                            in_=z32[:, 0:32].rearrange(
                                "p (a b c) -> p a b c", a=2, b=RG))
                        for c in range(2):
                            nc.sync.dma_start(
                                out=ft[:, c, :, 1:W + 1],
                                in_=feats_d[c * 128:(c + 1) * 128,
                                            gi * RG:(gi + 1) * RG, :]
                                .bitcast(f32r))
                        ftiles.append(ft)

                        # process row PAIRS whose input rows (y-1..y+2) are
                        # loaded: N=512 matmuls so LDWEIGHTS hides under the
                        # moving-operand stream.
                        if gi == 0:
                            pairs = [0, 2, 4]
                        elif gi == n_groups - 1:
                            pairs = [8 * gi - 2, 8 * gi, 8 * gi + 2,
                                     8 * gi + 4, 8 * gi + 6]
                        else:
                            pairs = [8 * gi - 2, 8 * gi, 8 * gi + 2, 8 * gi + 4]
                        for y in pairs:
                            acc = psA.tile([MID, 2, W], f32, name="acc")
                            mms = []  # (lhsT_sel, rhs_ap, out_ap)
                            # ky=1 first: always valid + full N=512, so the
                            # start=True matmul covers every PSUM element
                            for ky in (1, 0, 2):
                                for c in range(2):
                                    for kx in range(3):
                                        lw = (c, ky * 3 + kx)
                                        ys, ys2 = y + ky - 1, y + ky
                                        v0 = 0 <= ys < H
                                        v1 = 0 <= ys2 < H
                                        same = (v0 and v1
                                                and ys // RG == ys2 // RG)
                                        if same:
                                            src = ftiles[ys // RG]
                                            mms.append((lw,
                                                src[:, c, ys % RG:ys % RG + 2,
                                                    kx:kx + W],
                                                acc[:, :, :]))
                                        else:
                                            if v0:
                                                src = ftiles[ys // RG]
                                                mms.append((lw,
                                                    src[:, c, ys % RG, kx:kx + W],
                                                    acc[:, 0, :]))
                                            if v1:
                                                src = ftiles[ys2 // RG]
                                                mms.append((lw,
                                                    src[:, c, ys2 % RG, kx:kx + W],
                                                    acc[:, 1, :]))
                            for i, (lw, rhs, oap) in enumerate(mms):
                                nc.tensor.matmul(
                                    out=oap, lhsT=w1_r[:, lw[0], lw[1], :],
                                    rhs=rhs, start=(i == 0),
                                    stop=(i == len(mms) - 1))
                            # relu(x + bias) -> f32r (both rows, FD=512)
                            xr = xrp.tile([MID, 2, W], f32r, name="xr")
                            nc.scalar.activation(xr[:], acc[:], Act.Relu,
                                                 bias=bmid[:], scale=1.0)
                            accg = psG.tile([KD, 2, W], f32, name="accg")
                            nc.tensor.matmul(out=accg[:], lhsT=w2_r[:],
                                             rhs=xr[:], start=True, stop=True)
                            es = esp.tile([KD, 2, W], f32, name="es")
                            nc.scalar.activation(es[:], accg[:], Act.Exp,
                                                 bias=b2c[:], scale=1.0)
                            for r in range(2):
                                nc.sync.dma_start(
                                    out=e_all[y + r:y + r + 1, :].rearrange(
                                        "p (c w) -> p c w", c=KD),
                                    in_=es[:, r, :])

            # ================= softmax over 4 directions =================
            with tc.tile_pool(name="smx", bufs=1) as sp:
                s_all = sp.tile([128, K * W], f32)
                r_all = sp.tile([128, K * W], f32)
                ev = e_all[:].rearrange("p (k d w) -> p k d w", k=K, d=4)
                sv = s_all[:].rearrange("p (k w) -> p k w", k=K)
                nc.vector.tensor_tensor(out=sv, in0=ev[:, :, 0, :],
                                        in1=ev[:, :, 1, :], op=Alu.add)
                nc.vector.tensor_tensor(out=sv, in0=sv,
                                        in1=ev[:, :, 2, :], op=Alu.add)
                nc.vector.tensor_tensor(out=sv, in0=sv,
                                        in1=ev[:, :, 3, :], op=Alu.add)
                rv = r_all[:].rearrange("p (k w) -> p k w", k=K)
                nc.vector.reciprocal(out=r_all[:], in_=s_all[:])
                for d in range(4):
                    eng = nc.vector if d % 2 == 0 else nc.gpsimd
                    eng.tensor_tensor(out=ev[:, :, d, :], in0=ev[:, :, d, :],
                                      in1=rv, op=Alu.mult)

            # ---- load h0 = logits into guarded layout ----
            hv_a = h_a[:].rearrange("p (k w) -> p k w", k=K)
            hv_b = h_b[:].rearrange("p (k w) -> p k w", k=K)
            nc.vector.tensor_copy(
                out=hv_a[:, :, 0:WP:WP - 1],
                in_=z32[:, 0:2 * K].rearrange("p (k g) -> p k g", k=K))
            nc.vector.tensor_copy(
                out=hv_b[:, :, 0:WP:WP - 1],
                in_=z32[:, 0:2 * K].rearrange("p (k g) -> p k g", k=K))
            for k in range(K):
                nc.sync.dma_start(
                    out=h_a[:, k * WP + 1:k * WP + 1 + W],
                    in_=logits_d[k].bitcast(f32r))

            # ================= phase B: recurrence =================
            if True:
                with tc.tile_pool(name="tmp", bufs=4) as tp, \
                     tc.tile_pool(name="psS", bufs=3, space="PSUM") as psS:
                    cur, nxt = h_a, h_b
                    for t in range(T_STEPS):
                        for k in range(K):
                            base = k * WP
                            hwin = cur[:, base:base + WP]
                            up_ps = psS.tile([128, WP], f32, name="up_ps")
                            dn_ps = psS.tile([128, WP], f32, name="dn_ps")
                            nc.tensor.matmul(out=up_ps[:], lhsT=s_up[:],
                                             rhs=hwin, start=True, stop=True)
                            nc.tensor.matmul(out=dn_ps[:], lhsT=s_dn[:],
                                             rhs=hwin, start=True, stop=True)
                            gl = e_all[:, (4 * k + 0) * W:(4 * k + 1) * W]
                            gr = e_all[:, (4 * k + 1) * W:(4 * k + 2) * W]
                            gu = e_all[:, (4 * k + 2) * W:(4 * k + 3) * W]
                            gd = e_all[:, (4 * k + 3) * W:(4 * k + 4) * W]
                            left = cur[:, base:base + W].bitcast(f32)
                            right = cur[:, base + 2:base + 2 + W].bitcast(f32)
                            a = tp.tile([128, W], f32, name="a")
                            b = tp.tile([128, W], f32, name="b")
                            c2 = tp.tile([128, W], f32, name="c2")
                            d2 = tp.tile([128, W], f32, name="d2")
                            nc.vector.tensor_tensor(out=a[:], in0=gl, in1=left,
                                                    op=Alu.mult)
                            nc.gpsimd.tensor_tensor(out=b[:], in0=gr, in1=right,
                                                    op=Alu.mult)
                            nc.vector.tensor_tensor(out=c2[:], in0=gu,
                                                    in1=up_ps[:, 1:W + 1],
                                                    op=Alu.mult)
                            nc.vector.tensor_tensor(out=d2[:], in0=gd,
                                                    in1=dn_ps[:, 1:W + 1],
                                                    op=Alu.mult)
                            nc.gpsimd.tensor_tensor(out=a[:], in0=a[:], in1=b[:],
                                                    op=Alu.add)
                            nc.vector.tensor_tensor(out=c2[:], in0=c2[:],
                                                    in1=d2[:], op=Alu.add)
                            nc.vector.tensor_tensor(
                                out=nxt[:, base + 1:base + 1 + W],
                                in0=a[:], in1=c2[:], op=Alu.add)
                        cur, nxt = nxt, cur

                    for k in range(K):
                        nc.sync.dma_start(
                            out=out_d[k],
                            in_=cur[:, k * WP + 1:k * WP + 1 + W].bitcast(f32))

    nc.compile()
    return nc


_NC_CACHE = None


def kernel(feats, logits, w1, gamma, beta, mean, var, w2, b2):
    global _NC_CACHE
    from concourse.bass_utils import run_bass_kernel_spmd

    feats = np.asarray(feats, dtype=np.float32)
    logits = np.asarray(logits, dtype=np.float32)
    w1 = np.asarray(w1, dtype=np.float32)
    w2 = np.asarray(w2, dtype=np.float32)
    b2 = np.asarray(b2, dtype=np.float32)
    gamma = np.asarray(gamma, dtype=np.float32)
    beta = np.asarray(beta, dtype=np.float32)
    mean = np.asarray(mean, dtype=np.float32)
    var = np.asarray(var, dtype=np.float32)

    inv = gamma / np.sqrt(var + EPS)
    w1f = (w1 * inv[:, None, None, None]).astype(np.float32)  # [MID,CIN,3,3]
    bmid = (beta - mean * inv).astype(np.float32)[:, None]    # [MID,1]
    # [cin_in_chunk 128, chunk 2, tap 9, mid 128]
    w1t = (w1f.transpose(1, 2, 3, 0)                  # [CIN,3,3,MID]
           .reshape(2, 128, 9, MID)
           .transpose(1, 0, 2, 3)).copy()
    w2t = w2.reshape(KD, MID).T.copy()                # [MID,KD]
    b2c = b2[:, None].copy()
    s_up = np.eye(128, k=1, dtype=np.float32)         # out[m]=h[m-1]
    s_dn = np.eye(128, k=-1, dtype=np.float32)        # out[m]=h[m+1]

    if _NC_CACHE is None:
        _NC_CACHE = _build()
    nc = _NC_CACHE

    in_maps = []
    for i in range(B):
        in_maps.append({
            "feats": np.ascontiguousarray(feats[i]),
            "logits": np.ascontiguousarray(logits[i]),
            "w1t": w1t, "bmid": bmid, "w2t": w2t, "b2": b2c,
            "sup": s_up, "sdn": s_dn,
        })

    trace = bool(os.environ.get("KTRACE"))
    res = run_bass_kernel_spmd(nc, in_maps, list(range(B)), trace=trace)
    if trace and res.exec_time_ns is not None:
        print(f"HW exec time: {res.exec_time_ns} ns")
    out = np.stack([res.results[i]["out"] for i in range(B)], axis=0)
    return out.astype(np.float32)


if __name__ == "__main__":
    rng = np.random.default_rng(0)
    ins = {
        "feats": rng.standard_normal((B, CIN, H, W), dtype=np.float32),
        "logits": rng.standard_normal((B, K, H, W), dtype=np.float32),
        "w1": rng.standard_normal((MID, CIN, 3, 3), dtype=np.float32) / 48.0,
        "gamma": rng.standard_normal(MID).astype(np.float32) * 0.1 + 1.0,
        "beta": rng.standard_normal(MID).astype(np.float32) * 0.1,
        "mean": rng.standard_normal(MID).astype(np.float32) * 0.1,
        "var": rng.random(MID).astype(np.float32) + 0.5,
        "w2": rng.standard_normal((KD, MID, 1, 1)).astype(np.float32) / 11.3,
        "b2": rng.standard_normal(KD).astype(np.float32) * 0.01,
    }
    o = kernel(**ins)
    print("kernel out", o.shape, o.dtype, np.abs(o).mean())



# revision 2
# speedup vs baseline: 1.5776x; 1.5776x over previous
"""MCSPN Trainium2 kernel: guidance convs + softmax gates + 4-step CSPN recurrence.

Data-parallel over batch: 8 images -> 8 NeuronCores, one image per core.
fp16 everywhere (PSUM accum stays f32): halves feats DMA, 2x DVE, full-rate
matmuls at any N.

Per core:
  phase A: conv3x3 as 18 accumulating fp16 matmuls per row-pair, weight-cycled
           over groups of 3 row-pairs (1 LDWEIGHTS per tap per group); feats
           tiles are UNGUARDED/contiguous (4KB DMA packets); horizontal taps
           use edge-trimmed windows instead of guard columns. relu+BN bias on
           ACT -> conv1x1 (d-major output channels) -> exp on ACT -> per-row
           DMA scatter into gate planes e_all [y=128 part, (d,k,x) free].
  softmax: batched adds + fast-reciprocal + normalize mults over [128, 4864];
           then pre-shift gu/gd planes along y (PE shift-matmul + ACT copyback)
           and gl/gr are consumed via +-1 flat views (free).
  phase B: packed h [128, K*W]; per step: 4 gate-product mults (DVE/Pool),
           4 shift matmuls per 2-k chunk accumulating into PSUM (up/down via
           sub/super-diagonal, left/right via identity over shifted q windows),
           ACT evacuates PSUM -> nxt (fp16). No guards anywhere.
"""
import os
import sys

sys.path.insert(0, "/opt/trn_rl_repo")

import numpy as np

B, CIN, H, W = 8, 256, 128, 256
K = 19
MID = 128
KD = 4 * K  # 76
EPS = 1e-5
T_STEPS = 4
KW = K * W  # 4864
RG = 8      # feats rows per DMA group
WG = 3      # row-pairs per weight-cycle group


def _build():
    import concourse.bacc as bacc
    import concourse.mybir as mybir
    import concourse.tile as tile
    from concourse import bass

    f32 = mybir.dt.float32
    f16 = mybir.dt.float16
    Act = mybir.ActivationFunctionType
    Alu = mybir.AluOpType

    nc = bacc.Bacc("TRN2", target_bir_lowering=False)

    feats_d = nc.dram_tensor("feats", [CIN, H, W], f16, kind="ExternalInput")
    logits_d = nc.dram_tensor("logits", [H, K, W], f16, kind="ExternalInput")
    w1t_d = nc.dram_tensor("w1t", [128, 2, 9, MID], f16, kind="ExternalInput")
    bmid_d = nc.dram_tensor("bmid", [MID, 1], f32, kind="ExternalInput")
    w2t_d = nc.dram_tensor("w2t", [MID, KD], f16, kind="ExternalInput")
    b2_d = nc.dram_tensor("b2", [KD, 1], f32, kind="ExternalInput")
    sup_d = nc.dram_tensor("sup", [128, 128], f16, kind="ExternalInput")
    sdn_d = nc.dram_tensor("sdn", [128, 128], f16, kind="ExternalInput")
    idn_d = nc.dram_tensor("idn", [128, 128], f16, kind="ExternalInput")
    out_d = nc.dram_tensor("out", [H, K, W], f16, kind="ExternalOutput")

    with tile.TileContext(nc) as tc:
        with tc.tile_pool(name="persist", bufs=1) as pp, \
             tc.tile_pool(name="hpool", bufs=1) as hp:
            e_all = pp.tile([128, 4 * KW], f16)       # gate planes, d-major
            h_a = hp.tile([128, KW], f16)
            h_b = hp.tile([128, KW], f16)
            w2c = pp.tile([MID, KD], f16)
            bmid = pp.tile([MID, 1], f32)
            b2c = pp.tile([KD, 1], f32)
            s_up = pp.tile([128, 128], f16)           # out[p] = v[p-1]
            s_dn = pp.tile([128, 128], f16)           # out[p] = v[p+1]
            iden = pp.tile([128, 128], f16)

            nc.sync.dma_start(out=h_a[:], in_=logits_d[:, :, :])
            nc.sync.dma_start(out=bmid[:], in_=bmid_d[:])
            nc.sync.dma_start(out=b2c[:], in_=b2_d[:])
            nc.sync.dma_start(out=w2c[:], in_=w2t_d[:])
            nc.sync.dma_start(out=s_up[:], in_=sup_d[:])
            nc.sync.dma_start(out=s_dn[:], in_=sdn_d[:])
            nc.sync.dma_start(out=iden[:], in_=idn_d[:])

            # ================= phase A: guidance =================
            with tc.tile_pool(name="w1p", bufs=1) as w1p:
                w1 = w1p.tile([128, 2, 9, MID], f16)
                nc.sync.dma_start(out=w1[:], in_=w1t_d[:])

                with tc.tile_pool(name="frows", bufs=4) as frp, \
                     tc.tile_pool(name="xrow", bufs=3) as xrp, \
                     tc.tile_pool(name="estrip", bufs=3) as esp, \
                     tc.tile_pool(name="psA", bufs=6, space="PSUM") as psA, \
                     tc.tile_pool(name="psG", bufs=2, space="PSUM") as psG:
                    n_groups = H // RG
                    ftiles = {}

                    def load_group(g):
                        ft = frp.tile([128, 2, RG, W], f16, name=f"ft{g}",
                                      tag="ft")
                        for c in range(2):
                            nc.sync.dma_start(
                                out=ft[:, c],
                                in_=feats_d[c * 128:(c + 1) * 128,
                                            g * RG:(g + 1) * RG, :])
                        ftiles[g] = ft

                    # tap order: full-coverage ky=1 taps first & last so the
                    # start/stop matmuls cover every PSUM element.
                    taps = [(0, 1, 1)]
                    for c in range(2):
                        for ky in range(3):
                            for kx in range(3):
                                if (c, ky, kx) not in ((0, 1, 1), (1, 1, 1)):
                                    taps.append((c, ky, kx))
                    taps.append((1, 1, 1))

                    def emit_taps(wg, accs):
                        for ti, (c, ky, kx) in enumerate(taps):
                            lw = w1[:, c, ky * 3 + kx, :]
                            first = ti == 0
                            last = ti == len(taps) - 1
                            for y in wg:
                                acc = accs[y]
                                rows = [(r, y + r + ky - 1) for r in range(2)
                                        if 0 <= y + r + ky - 1 < H]
                                mms = []
                                if (len(rows) == 2
                                        and rows[0][1] // RG == rows[1][1] // RG):
                                    g, ro = rows[0][1] // RG, rows[0][1] % RG
                                    mms.append((ftiles[g][:, c, ro:ro + 2, :],
                                                acc[:, 0:2, :]))
                                else:
                                    for (r, yin) in rows:
                                        g, ro = yin // RG, yin % RG
                                        mms.append((ftiles[g][:, c, ro, :],
                                                    acc[:, r, :]))
                                for rhs_full, oap in mms:
                                    if kx == 0:
                                        rhs = rhs_full[..., 0:W - 1]
                                        oap = oap[..., 1:W]
                                    elif kx == 2:
                                        rhs = rhs_full[..., 1:W]
                                        oap = oap[..., 0:W - 1]
                                    else:
                                        rhs = rhs_full
                                    nc.tensor.matmul(out=oap, lhsT=lw, rhs=rhs,
                                                     start=first, stop=last)

                    def emit_post(wg, accs):
                        for y in wg:
                            xr = xrp.tile([MID, 2, W], f16, name="xr")
                            nc.scalar.activation(xr[:], accs[y][:], Act.Relu,
                                                 bias=bmid[:], scale=1.0)
                            accg = psG.tile([KD, 2, W], f32, name="accg")
                            nc.tensor.matmul(out=accg[:], lhsT=w2c[:],
                                             rhs=xr[:], start=True, stop=True)
                            es = esp.tile([KD, 2, W], f16, name="es")
                            nc.scalar.activation(es[:], accg[:], Act.Exp,
                                                 bias=b2c[:], scale=1.0)
                            for r in range(2):
                                nc.sync.dma_start(
                                    out=e_all[y + r:y + r + 1, :].rearrange(
                                        "p (c x) -> p c x", c=KD),
                                    in_=es[:, r, :])

                    pairs = list(range(0, H, 2))
                    wgs = [pairs[i:i + WG] for i in range(0, len(pairs), WG)]
                    emitted = 0

                    def ensure_groups(upto):
                        nonlocal emitted
                        while emitted < min(upto, n_groups):
                            load_group(emitted)
                            emitted += 1

                    all_accs = {}
                    for i, wg in enumerate(wgs):
                        ensure_groups((wg[-1] + 2) // RG + 2)
                        for y in wg:
                            all_accs[y] = psA.tile([MID, 2, W], f32,
                                                   name=f"acc{y}", tag="acc")
                        emit_taps(wg, all_accs)
                        if i > 0:
                            emit_post(wgs[i - 1], all_accs)
                    emit_post(wgs[-1], all_accs)

            # ================= softmax + gate pre-shift =================
            pl = [e_all[:, d * KW:(d + 1) * KW] for d in range(4)]
            with tc.tile_pool(name="smx", bufs=1) as sp, \
                 tc.tile_pool(name="psP", bufs=4, space="PSUM") as psP:
                a01 = sp.tile([128, KW], f16)
                a23 = sp.tile([128, KW], f16)
                sf = sp.tile([128, KW], f32)
                rf = sp.tile([128, KW], f32)
                r16 = sp.tile([128, KW], f16)
                nc.vector.tensor_tensor(out=a01[:], in0=pl[0], in1=pl[1],
                                        op=Alu.add)
                nc.gpsimd.tensor_tensor(out=a23[:], in0=pl[2], in1=pl[3],
                                        op=Alu.add)
                nc.vector.tensor_tensor(out=sf[:], in0=a01[:], in1=a23[:],
                                        op=Alu.add)
                nc.vector.reciprocal_approx_fast(out=rf[:], in_=sf[:])
                nc.scalar.activation(r16[:], rf[:], Act.Copy)
                # normalize: gu/gd first (pre-shift depends on them)
                nc.vector.tensor_tensor(out=pl[2], in0=pl[2], in1=r16[:],
                                        op=Alu.mult)
                nc.vector.tensor_tensor(out=pl[3], in0=pl[3], in1=r16[:],
                                        op=Alu.mult)
                # pre-shift gu/gd along y, in place via PSUM bounce:
                # gu'[y] = gu[y+1] (lhsT=s_dn) ; gd'[y] = gd[y-1] (lhsT=s_up)
                for d, mat in ((2, s_dn), (3, s_up)):
                    for k0 in range(0, K, 2):
                        ck = min(2, K - k0)
                        src = e_all[:, d * KW + k0 * W:
                                    d * KW + (k0 + ck) * W]
                        pps = psP.tile([128, 2 * W], f32, name="pps")
                        nc.tensor.matmul(out=pps[:, 0:ck * W], lhsT=mat[:],
                                         rhs=src, start=True, stop=True)
                        nc.scalar.activation(src, pps[:, 0:ck * W], Act.Copy)
                nc.vector.tensor_tensor(out=pl[0], in0=pl[0], in1=r16[:],
                                        op=Alu.mult)
                nc.gpsimd.tensor_tensor(out=pl[1], in0=pl[1], in1=r16[:],
                                        op=Alu.mult)

            # ================= phase B: recurrence =================
            thirds = [(0, 8), (8, 16), (16, 19)]
            with tc.tile_pool(name="qp", bufs=1) as qp, \
                 tc.tile_pool(name="psS", bufs=2, space="PSUM") as psS:
                q_u = qp.tile([128, KW], f16)
                q_d = qp.tile([128, KW], f16)
                q_l = qp.tile([128, KW], f16)
                q_r = qp.tile([128, KW], f16)
                cur, nxt = h_a, h_b
                for t in range(T_STEPS):
                    for (k0, k1) in thirds:
                        nk = k1 - k0
                        f0, f1 = k0 * W, k1 * W
                        hseg = cur[:, f0:f1]
                        # gate products; gl/gr consumed via +-1 flat views
                        nc.vector.tensor_tensor(
                            out=q_u[:, f0:f1], in0=e_all[:, 2 * KW + f0:
                                                         2 * KW + f1],
                            in1=hseg, op=Alu.mult)
                        nc.vector.tensor_tensor(
                            out=q_d[:, f0:f1], in0=e_all[:, 3 * KW + f0:
                                                         3 * KW + f1],
                            in1=hseg, op=Alu.mult)
                        nc.vector.tensor_tensor(
                            out=q_l[:, f0:f1], in0=e_all[:, f0 + 1:f1 + 1],
                            in1=hseg, op=Alu.mult)
                        nc.gpsimd.tensor_tensor(
                            out=q_r[:, f0:f1], in0=e_all[:, KW + f0 - 1:
                                                         KW + f1 - 1],
                            in1=hseg, op=Alu.mult)
                        ps = psS.tile([128, 8, W], f32, name="ps")
                        chunks = [(a, min(a + 2, nk)) for a in range(0, nk, 2)]
                        for (a, b) in chunks:
                            nc.tensor.matmul(
                                out=ps[:, a:b, :], lhsT=s_up[:],
                                rhs=q_u[:, f0 + a * W:f0 + b * W],
                                start=True, stop=False)
                        for (a, b) in chunks:
                            nc.tensor.matmul(
                                out=ps[:, a:b, :], lhsT=s_dn[:],
                                rhs=q_d[:, f0 + a * W:f0 + b * W],
                                start=False, stop=False)
                        for (a, b) in chunks:
                            # agg[x] += q_l[x-1] for x>=1
                            nc.tensor.matmul(
                                out=ps[:, a:b, 1:W], lhsT=iden[:],
                                rhs=q_l[:, f0 + a * W:f0 + b * W].rearrange(
                                    "p (k x) -> p k x", k=b - a)[:, :, 0:W - 1],
                                start=False, stop=False)
                        for (a, b) in chunks:
                            # agg[x] += q_r[x+1] for x<W-1
                            nc.tensor.matmul(
                                out=ps[:, a:b, 0:W - 1], lhsT=iden[:],
                                rhs=q_r[:, f0 + a * W:f0 + b * W].rearrange(
                                    "p (k x) -> p k x", k=b - a)[:, :, 1:W],
                                start=False, stop=True)
                        nc.scalar.activation(nxt[:, f0:f1], ps[:, 0:nk, :],
                                             Act.Copy)
                    cur, nxt = nxt, cur

                nc.sync.dma_start(out=out_d[:, :, :],
                                  in_=cur[:].rearrange("p (k x) -> p k x", k=K))

    nc.compile()
    return nc


_NC_CACHE = None


def kernel(feats, logits, w1, gamma, beta, mean, var, w2, b2):
    global _NC_CACHE
    from concourse.bass_utils import run_bass_kernel_spmd

    feats = np.asarray(feats, dtype=np.float32)
    logits = np.asarray(logits, dtype=np.float32)
    w1 = np.asarray(w1, dtype=np.float32)
    w2 = np.asarray(w2, dtype=np.float32)
    b2 = np.asarray(b2, dtype=np.float32)
    gamma = np.asarray(gamma, dtype=np.float32)
    beta = np.asarray(beta, dtype=np.float32)
    mean = np.asarray(mean, dtype=np.float32)
    var = np.asarray(var, dtype=np.float32)

    inv = gamma / np.sqrt(var + EPS)
    w1f = (w1 * inv[:, None, None, None]).astype(np.float32)  # [MID,CIN,3,3]
    bmid = (beta - mean * inv).astype(np.float32)[:, None]    # [MID,1]
    # [cin_in_chunk 128, chunk 2, tap 9, mid 128]
    w1t = (w1f.transpose(1, 2, 3, 0)                  # [CIN,3,3,MID]
           .reshape(2, 128, 9, MID)
           .transpose(1, 0, 2, 3)).astype(np.float16).copy()
    # d-major output channel order: new channel p = d*K + k <- old k*4 + d
    perm = np.array([k * 4 + d for d in range(4) for k in range(K)])
    w2t = w2.reshape(KD, MID)[perm].T.astype(np.float16).copy()  # [MID,KD]
    b2c = b2[perm][:, None].astype(np.float32).copy()
    s_up = np.eye(128, k=1, dtype=np.float16)         # out[m]=v[m-1]
    s_dn = np.eye(128, k=-1, dtype=np.float16)        # out[m]=v[m+1]
    idn = np.eye(128, dtype=np.float16)

    if _NC_CACHE is None:
        _NC_CACHE = _build()
    nc = _NC_CACHE

    in_maps = []
    for i in range(B):
        in_maps.append({
            "feats": np.ascontiguousarray(feats[i]).astype(np.float16),
            "logits": np.ascontiguousarray(
                logits[i].transpose(1, 0, 2)).astype(np.float16),
            "w1t": w1t, "bmid": bmid, "w2t": w2t, "b2": b2c,
            "sup": s_up, "sdn": s_dn, "idn": idn,
        })

    trace = bool(os.environ.get("KTRACE"))
    res = run_bass_kernel_spmd(nc, in_maps, list(range(B)), trace=trace)
    if trace and res.exec_time_ns is not None:
        print(f"HW exec time: {res.exec_time_ns} ns")
    out = np.stack([res.results[i]["out"] for i in range(B)], axis=0)
    return out.transpose(0, 2, 1, 3).astype(np.float32)


if __name__ == "__main__":
    rng = np.random.default_rng(0)
    ins = {
        "feats": rng.standard_normal((B, CIN, H, W), dtype=np.float32),
        "logits": rng.standard_normal((B, K, H, W), dtype=np.float32),
        "w1": rng.standard_normal((MID, CIN, 3, 3), dtype=np.float32) / 48.0,
        "gamma": rng.standard_normal(MID).astype(np.float32) * 0.1 + 1.0,
        "beta": rng.standard_normal(MID).astype(np.float32) * 0.1,
        "mean": rng.standard_normal(MID).astype(np.float32) * 0.1,
        "var": rng.random(MID).astype(np.float32) + 0.5,
        "w2": rng.standard_normal((KD, MID, 1, 1)).astype(np.float32) / 11.3,
        "b2": rng.standard_normal(KD).astype(np.float32) * 0.01,
    }
    o = kernel(**ins)
    print("kernel out", o.shape, o.dtype, np.abs(o).mean())


# revision 11
# speedup vs baseline: 1.7645x; 1.1185x over previous
"""MCSPN Trainium2 kernel: guidance convs + softmax gates + 4-step CSPN recurrence.

Data-parallel over batch: 8 images -> 8 NeuronCores, one image per core.
fp16 everywhere (PSUM accum stays f32): halves feats DMA, 2x DVE, full-rate
matmuls at any N.

Per core:
  phase A: conv3x3 as 18 accumulating fp16 matmuls per row-pair, weight-cycled
           over groups of 3 row-pairs (1 LDWEIGHTS per tap per group); feats
           tiles are UNGUARDED/contiguous (4KB DMA packets); horizontal taps
           use edge-trimmed windows instead of guard columns. relu+BN bias on
           ACT -> conv1x1 (d-major output channels) -> exp on ACT -> per-row
           DMA scatter into gate planes e_all [y=128 part, (d,k,x) free].
  softmax: batched adds + fast-reciprocal + normalize mults over [128, 4864];
           then pre-shift gu/gd planes along y (PE shift-matmul + ACT copyback)
           and gl/gr are consumed via +-1 flat views (free).
  phase B: packed h [128, K*W]; per step: 4 gate-product mults (DVE/Pool),
           4 shift matmuls per 2-k chunk accumulating into PSUM (up/down via
           sub/super-diagonal, left/right via identity over shifted q windows),
           ACT evacuates PSUM -> nxt (fp16). No guards anywhere.
"""
import os
import sys

sys.path.insert(0, "/opt/trn_rl_repo")

import numpy as np

B, CIN, H, W = 8, 256, 128, 256
K = 19
MID = 128
KD = 4 * K  # 76
EPS = 1e-5
T_STEPS = 4
KW = K * W  # 4864
RG = 8      # feats rows per DMA group
WG = 2      # row-pairs per weight-cycle group


def _build():
    import concourse.bacc as bacc
    import concourse.mybir as mybir
    import concourse.tile as tile
    from concourse import bass

    f32 = mybir.dt.float32
    f16 = mybir.dt.float16
    Act = mybir.ActivationFunctionType
    Alu = mybir.AluOpType

    nc = bacc.Bacc("TRN2", target_bir_lowering=False)

    feats_d = nc.dram_tensor("feats", [CIN, H, W], f16, kind="ExternalInput")
    logits_d = nc.dram_tensor("logits", [H, K, W], f16, kind="ExternalInput")
    w1t_d = nc.dram_tensor("w1t", [128, 2, 9, MID], f16, kind="ExternalInput")
    bmid_d = nc.dram_tensor("bmid", [MID, 1], f32, kind="ExternalInput")
    w2t_d = nc.dram_tensor("w2t", [MID, KD], f16, kind="ExternalInput")
    b2_d = nc.dram_tensor("b2", [KD, 1], f32, kind="ExternalInput")
    sup_d = nc.dram_tensor("sup", [128, 128], f16, kind="ExternalInput")
    sdn_d = nc.dram_tensor("sdn", [128, 128], f16, kind="ExternalInput")
    idn_d = nc.dram_tensor("idn", [128, 128], f16, kind="ExternalInput")
    out_d = nc.dram_tensor("out", [H, K, W], f16, kind="ExternalOutput")

    with tile.TileContext(nc) as tc:
        with tc.tile_pool(name="persist", bufs=1) as pp, \
             tc.tile_pool(name="hpool", bufs=1) as hp:
            e_all = pp.tile([128, 4 * KW], f16)       # gate planes, d-major
            h_a = hp.tile([128, KW], f16)
            h_b = hp.tile([128, KW], f16)
            w2c = pp.tile([MID, KD], f16)
            bmid = pp.tile([MID, 1], f32)
            b2c = pp.tile([KD, 1], f32)
            s_up = pp.tile([128, 128], f16)           # out[p] = v[p-1]
            s_dn = pp.tile([128, 128], f16)           # out[p] = v[p+1]
            iden = pp.tile([128, 128], f16)

            nc.sync.dma_start(out=bmid[:], in_=bmid_d[:])
            nc.sync.dma_start(out=b2c[:], in_=b2_d[:])
            nc.sync.dma_start(out=w2c[:], in_=w2t_d[:])
            nc.sync.dma_start(out=s_up[:], in_=sup_d[:])
            nc.sync.dma_start(out=s_dn[:], in_=sdn_d[:])
            nc.sync.dma_start(out=iden[:], in_=idn_d[:])

            # ================= phase A: guidance =================
            with tc.tile_pool(name="w1p", bufs=1) as w1p:
                w1 = w1p.tile([128, 2, 9, MID], f16)
                # per-chunk loads so chunk-0 taps can start sooner
                nc.sync.dma_start(out=w1[:, 0], in_=w1t_d[:, 0])
                nc.sync.dma_start(out=w1[:, 1], in_=w1t_d[:, 1])

                with tc.tile_pool(name="frows", bufs=4) as frp, \
                     tc.tile_pool(name="xrow", bufs=3) as xrp, \
                     tc.tile_pool(name="estrip", bufs=3) as esp, \
                     tc.tile_pool(name="psA", bufs=4, space="PSUM") as psA, \
                     tc.tile_pool(name="psG", bufs=3, space="PSUM") as psG:
                    n_groups = H // RG
                    ftiles = {}

                    def load_group(g):
                        ft = frp.tile([128, 2, RG, W], f16, name=f"ft{g}",
                                      tag="ft")
                        for c in range(2):
                            nc.sync.dma_start(
                                out=ft[:, c],
                                in_=feats_d[c * 128:(c + 1) * 128,
                                            g * RG:(g + 1) * RG, :])
                        ftiles[g] = ft

                    # tap order: full-coverage ky=1 taps first & last so the
                    # start/stop matmuls cover every PSUM element.
                    taps = [(0, 1, 1)]
                    for c in range(2):
                        for ky in range(3):
                            for kx in range(3):
                                if (c, ky, kx) not in ((0, 1, 1), (1, 1, 1)):
                                    taps.append((c, ky, kx))
                    taps.append((1, 1, 1))

                    def emit_taps(wg, accs):
                        for ti, (c, ky, kx) in enumerate(taps):
                            lw = w1[:, c, ky * 3 + kx, :]
                            first = ti == 0
                            last = ti == len(taps) - 1
                            for y in wg:
                                acc = accs[y]
                                rows = [(r, y + r + ky - 1) for r in range(2)
                                        if 0 <= y + r + ky - 1 < H]
                                mms = []
                                if (len(rows) == 2
                                        and rows[0][1] // RG == rows[1][1] // RG):
                                    g, ro = rows[0][1] // RG, rows[0][1] % RG
                                    mms.append((ftiles[g][:, c, ro:ro + 2, :],
                                                acc[:, 0:2, :]))
                                else:
                                    for (r, yin) in rows:
                                        g, ro = yin // RG, yin % RG
                                        mms.append((ftiles[g][:, c, ro, :],
                                                    acc[:, r, :]))
                                for rhs_full, oap in mms:
                                    if kx == 0:
                                        rhs = rhs_full[..., 0:W - 1]
                                        oap = oap[..., 1:W]
                                    elif kx == 2:
                                        rhs = rhs_full[..., 1:W]
                                        oap = oap[..., 0:W - 1]
                                    else:
                                        rhs = rhs_full
                                    nc.tensor.matmul(out=oap, lhsT=lw, rhs=rhs,
                                                     start=first, stop=last)

                    def emit_post(wg, accs):
                        # all relus first: ACT's strict FIFO must not queue an
                        # exp (which waits on a conv1x1) ahead of a relu the
                        # conv1x1s need -- that convoy stalls the PE.
                        xrs, accgs = {}, {}
                        for y in wg:
                            xr = xrp.tile([MID, 2, W], f16, name="xr")
                            nc.scalar.activation(xr[:], accs[y][:], Act.Relu,
                                                 bias=bmid[:], scale=1.0)
                            xrs[y] = xr
                        for y in wg:
                            accg = psG.tile([KD, 2, W], f32, name="accg")
                            nc.tensor.matmul(out=accg[:], lhsT=w2c[:],
                                             rhs=xrs[y][:], start=True,
                                             stop=True)
                            accgs[y] = accg
                        for y in wg:
                            es = esp.tile([KD, 2, W], f16, name="es")
                            nc.scalar.activation(es[:], accgs[y][:], Act.Exp,
                                                 bias=b2c[:], scale=1.0)
                            for r in range(2):
                                nc.sync.dma_start(
                                    out=e_all[y + r:y + r + 1, :].rearrange(
                                        "p (c x) -> p c x", c=KD),
                                    in_=es[:, r, :])

                    pairs = list(range(0, H, 2))
                    wgs = [pairs[i:i + WG] for i in range(0, len(pairs), WG)]
                    emitted = 0

                    def ensure_groups(upto):
                        nonlocal emitted
                        while emitted < min(upto, n_groups):
                            load_group(emitted)
                            emitted += 1

                    all_accs = {}
                    for i, wg in enumerate(wgs):
                        ensure_groups((wg[-1] + 2) // RG + 2)
                        for y in wg:
                            all_accs[y] = psA.tile([MID, 2, W], f32,
                                                   name=f"acc{y}", tag="acc")
                        emit_taps(wg, all_accs)
                        if i > 0:
                            emit_post(wgs[i - 1], all_accs)
                        if i == 3:
                            # h0 load, placed away from the startup DMA burst
                            nc.sync.dma_start(out=h_a[:],
                                              in_=logits_d[:, :, :])
                    emit_post(wgs[-1], all_accs)

            # ================= softmax + gate pre-shift =================
            pl = [e_all[:, d * KW:(d + 1) * KW] for d in range(4)]
            with tc.tile_pool(name="smx", bufs=1) as sp, \
                 tc.tile_pool(name="psP", bufs=4, space="PSUM") as psP:
                a01 = sp.tile([128, KW], f16)
                a23 = sp.tile([128, KW], f16)
                sf = sp.tile([128, KW], f32)
                rf = sp.tile([128, KW], f32)
                r16 = sp.tile([128, KW], f16)
                nc.vector.tensor_tensor(out=a01[:], in0=pl[0], in1=pl[1],
                                        op=Alu.add)
                nc.vector.tensor_tensor(out=a23[:], in0=pl[2], in1=pl[3],
                                        op=Alu.add)
                nc.vector.tensor_tensor(out=sf[:], in0=a01[:], in1=a23[:],
                                        op=Alu.add)
                nc.vector.reciprocal_approx_fast(out=rf[:], in_=sf[:])
                nc.scalar.activation(r16[:], rf[:], Act.Copy)
                # normalize: gu/gd first (pre-shift depends on them)
                nc.vector.tensor_tensor(out=pl[2], in0=pl[2], in1=r16[:],
                                        op=Alu.mult)
                nc.vector.tensor_tensor(out=pl[3], in0=pl[3], in1=r16[:],
                                        op=Alu.mult)
                # pre-shift gu/gd along y, in place via PSUM bounce:
                # gu'[y] = gu[y+1] (lhsT=s_dn) ; gd'[y] = gd[y-1] (lhsT=s_up)
                for d, mat in ((2, s_dn), (3, s_up)):
                    for k0 in range(0, K, 2):
                        ck = min(2, K - k0)
                        src = e_all[:, d * KW + k0 * W:
                                    d * KW + (k0 + ck) * W]
                        pps = psP.tile([128, 2 * W], f32, name="pps")
                        nc.tensor.matmul(out=pps[:, 0:ck * W], lhsT=mat[:],
                                         rhs=src, start=True, stop=True)
                        nc.scalar.activation(src, pps[:, 0:ck * W], Act.Copy)
                # gr: 70% on DVE, 30% on Pool (Pool has no fp16 speedup)
                cut = 13 * W
                nc.vector.tensor_tensor(out=pl[0], in0=pl[0], in1=r16[:],
                                        op=Alu.mult)
                nc.vector.tensor_tensor(
                    out=e_all[:, KW:KW + cut], in0=e_all[:, KW:KW + cut],
                    in1=r16[:, 0:cut], op=Alu.mult)
                nc.gpsimd.tensor_tensor(
                    out=e_all[:, KW + cut:2 * KW],
                    in0=e_all[:, KW + cut:2 * KW],
                    in1=r16[:, cut:KW], op=Alu.mult)

            # ================= phase B: recurrence =================
            # small third first, with its q_u/q_d on Pool: Pool's slow ops run
            # concurrently with DVE's work on the two big thirds.
            thirds = [(16, 19), (0, 8), (8, 16)]
            with tc.tile_pool(name="qp", bufs=1) as qp, \
                 tc.tile_pool(name="psS", bufs=2, space="PSUM") as psS:
                q_u = qp.tile([128, KW], f16)
                q_d = qp.tile([128, KW], f16)
                q_l = qp.tile([128, KW], f16)
                q_r = qp.tile([128, KW], f16)
                cur, nxt = h_a, h_b
                for t in range(T_STEPS):
                    for (k0, k1) in thirds:
                        nk = k1 - k0
                        f0, f1 = k0 * W, k1 * W
                        hseg = cur[:, f0:f1]
                        small = nk < 8
                        eu = nc.gpsimd if small else nc.vector
                        # gate products; gl/gr consumed via +-1 flat views
                        eu.tensor_tensor(
                            out=q_u[:, f0:f1], in0=e_all[:, 2 * KW + f0:
                                                         2 * KW + f1],
                            in1=hseg, op=Alu.mult)
                        eu.tensor_tensor(
                            out=q_d[:, f0:f1], in0=e_all[:, 3 * KW + f0:
                                                         3 * KW + f1],
                            in1=hseg, op=Alu.mult)
                        nc.vector.tensor_tensor(
                            out=q_l[:, f0:f1], in0=e_all[:, f0 + 1:f1 + 1],
                            in1=hseg, op=Alu.mult)
                        nc.vector.tensor_tensor(
                            out=q_r[:, f0:f1], in0=e_all[:, KW + f0 - 1:
                                                         KW + f1 - 1],
                            in1=hseg, op=Alu.mult)
                        ps = psS.tile([128, 8, W], f32, name="ps")
                        chunks = [(a, min(a + 2, nk)) for a in range(0, nk, 2)]
                        for (a, b) in chunks:
                            nc.tensor.matmul(
                                out=ps[:, a:b, :], lhsT=s_up[:],
                                rhs=q_u[:, f0 + a * W:f0 + b * W],
                                start=True, stop=False)
                        for (a, b) in chunks:
                            nc.tensor.matmul(
                                out=ps[:, a:b, :], lhsT=s_dn[:],
                                rhs=q_d[:, f0 + a * W:f0 + b * W],
                                start=False, stop=False)
                        for (a, b) in chunks:
                            # agg[x] += q_l[x-1] for x>=1
                            nc.tensor.matmul(
                                out=ps[:, a:b, 1:W], lhsT=iden[:],
                                rhs=q_l[:, f0 + a * W:f0 + b * W].rearrange(
                                    "p (k x) -> p k x", k=b - a)[:, :, 0:W - 1],
                                start=False, stop=False)
                        for (a, b) in chunks:
                            # agg[x] += q_r[x+1] for x<W-1
                            nc.tensor.matmul(
                                out=ps[:, a:b, 0:W - 1], lhsT=iden[:],
                                rhs=q_r[:, f0 + a * W:f0 + b * W].rearrange(
                                    "p (k x) -> p k x", k=b - a)[:, :, 1:W],
                                start=False, stop=True)
                        nc.scalar.activation(nxt[:, f0:f1], ps[:, 0:nk, :],
                                             Act.Copy)
                    cur, nxt = nxt, cur

                nc.sync.dma_start(out=out_d[:, :, :],
                                  in_=cur[:].rearrange("p (k x) -> p k x", k=K))

    nc.compile()
    return nc


_NC_CACHE = None


def kernel(feats, logits, w1, gamma, beta, mean, var, w2, b2):
    global _NC_CACHE
    from concourse.bass_utils import run_bass_kernel_spmd

    feats = np.asarray(feats, dtype=np.float32)
    logits = np.asarray(logits, dtype=np.float32)
    w1 = np.asarray(w1, dtype=np.float32)
    w2 = np.asarray(w2, dtype=np.float32)
    b2 = np.asarray(b2, dtype=np.float32)
    gamma = np.asarray(gamma, dtype=np.float32)
    beta = np.asarray(beta, dtype=np.float32)
    mean = np.asarray(mean, dtype=np.float32)
    var = np.asarray(var, dtype=np.float32)

    inv = gamma / np.sqrt(var + EPS)
    w1f = (w1 * inv[:, None, None, None]).astype(np.float32)  # [MID,CIN,3,3]
    bmid = (beta - mean * inv).astype(np.float32)[:, None]    # [MID,1]
    # [cin_in_chunk 128, chunk 2, tap 9, mid 128]
    w1t = (w1f.transpose(1, 2, 3, 0)                  # [CIN,3,3,MID]
           .reshape(2, 128, 9, MID)
           .transpose(1, 0, 2, 3)).astype(np.float16).copy()
    # d-major output channel order: new channel p = d*K + k <- old k*4 + d
    perm = np.array([k * 4 + d for d in range(4) for k in range(K)])
    w2t = w2.reshape(KD, MID)[perm].T.astype(np.float16).copy()  # [MID,KD]
    b2c = b2[perm][:, None].astype(np.float32).copy()
    s_up = np.eye(128, k=1, dtype=np.float16)         # out[m]=v[m-1]
    s_dn = np.eye(128, k=-1, dtype=np.float16)        # out[m]=v[m+1]
    idn = np.eye(128, dtype=np.float16)

    if _NC_CACHE is None:
        _NC_CACHE = _build()
    nc = _NC_CACHE

    in_maps = []
    for i in range(B):
        in_maps.append({
            "feats": np.ascontiguousarray(feats[i]).astype(np.float16),
            "logits": np.ascontiguousarray(
                logits[i].transpose(1, 0, 2)).astype(np.float16),
            "w1t": w1t, "bmid": bmid, "w2t": w2t, "b2": b2c,
            "sup": s_up, "sdn": s_dn, "idn": idn,
        })

    trace = bool(os.environ.get("KTRACE"))
    res = run_bass_kernel_spmd(nc, in_maps, list(range(B)), trace=trace)
    if trace and res.exec_time_ns is not None:
        print(f"HW exec time: {res.exec_time_ns} ns")
    out = np.stack([res.results[i]["out"] for i in range(B)], axis=0)
    return out.transpose(0, 2, 1, 3).astype(np.float32)


if __name__ == "__main__":
    rng = np.random.default_rng(0)
    ins = {
        "feats": rng.standard_normal((B, CIN, H, W), dtype=np.float32),
        "logits": rng.standard_normal((B, K, H, W), dtype=np.float32),
        "w1": rng.standard_normal((MID, CIN, 3, 3), dtype=np.float32) / 48.0,
        "gamma": rng.standard_normal(MID).astype(np.float32) * 0.1 + 1.0,
        "beta": rng.standard_normal(MID).astype(np.float32) * 0.1,
        "mean": rng.standard_normal(MID).astype(np.float32) * 0.1,
        "var": rng.random(MID).astype(np.float32) + 0.5,
        "w2": rng.standard_normal((KD, MID, 1, 1)).astype(np.float32) / 11.3,
        "b2": rng.standard_normal(KD).astype(np.float32) * 0.01,
    }
    o = kernel(**ins)
    print("kernel out", o.shape, o.dtype, np.abs(o).mean())


# revision 12
# speedup vs baseline: 1.9386x; 1.0986x over previous
"""MCSPN Trainium2 kernel: guidance convs + fused softmax gates + 4-step CSPN.

Data-parallel over batch: 8 images -> 8 NeuronCores, one image per core.
fp16 everywhere (PSUM accum stays f32).

Per core:
  phase A: conv3x3 as 18 accumulating fp16 matmuls per row-pair, weight-cycled
           over groups of 2 row-pairs; feats tiles UNGUARDED/contiguous (4KB
           DMA packets); horizontal taps use edge-trimmed windows. Pipelined
           post-stages (one/two blocks behind the taps so no engine convoys):
           relu+BN (ACT) -> conv1x1 d-major (PE) -> exp (ACT) ->
           softmax fused in: partition-sum over the 4 directions via a
           tiled-eye matmul (PE) -> fast reciprocal + in-place normalize on
           the otherwise-idle DVE -> gate scatter DMAs issued from the idle
           GpSimd queue (keeps Sync free for feats loads).
  bridge:  pre-shift gu/gd gate planes along y (PE shift-matmul + ACT copy).
  phase B: packed h [128, K*W]; per step: 4 gate-product mults (DVE; GpSimd
           assists on the small k-chunk), 4 shift matmuls per 2-k chunk
           accumulating into PSUM (up/down via sub/super-diagonal, left/right
           via identity over shifted q windows), ACT evacuates PSUM -> nxt.
"""
import os
import sys

sys.path.insert(0, "/opt/trn_rl_repo")

import numpy as np

B, CIN, H, W = 8, 256, 128, 256
K = 19
MID = 128
KD = 4 * K  # 76
EPS = 1e-5
T_STEPS = 4
KW = K * W  # 4864
RG = 8      # feats rows per DMA group
WG = 2      # row-pairs per weight-cycle group


def _build():
    import concourse.bacc as bacc
    import concourse.mybir as mybir
    import concourse.tile as tile
    from concourse import bass

    f32 = mybir.dt.float32
    f16 = mybir.dt.float16
    Act = mybir.ActivationFunctionType
    Alu = mybir.AluOpType

    nc = bacc.Bacc("TRN2", target_bir_lowering=False)

    feats_d = nc.dram_tensor("feats", [CIN, H, W], f16, kind="ExternalInput")
    logits_d = nc.dram_tensor("logits", [H, K, W], f16, kind="ExternalInput")
    w1t_d = nc.dram_tensor("w1t", [128, 2, 9, MID], f16, kind="ExternalInput")
    bmid_d = nc.dram_tensor("bmid", [MID, 1], f32, kind="ExternalInput")
    w2t_d = nc.dram_tensor("w2t", [MID, KD], f16, kind="ExternalInput")
    b2_d = nc.dram_tensor("b2", [KD, 1], f32, kind="ExternalInput")
    sup_d = nc.dram_tensor("sup", [128, 128], f16, kind="ExternalInput")
    sdn_d = nc.dram_tensor("sdn", [128, 128], f16, kind="ExternalInput")
    idn_d = nc.dram_tensor("idn", [128, 128], f16, kind="ExternalInput")
    osum_d = nc.dram_tensor("osum", [KD, KD], f16, kind="ExternalInput")
    out_d = nc.dram_tensor("out", [H, K, W], f16, kind="ExternalOutput")

    with tile.TileContext(nc) as tc:
        with tc.tile_pool(name="persist", bufs=1) as pp, \
             tc.tile_pool(name="hpool", bufs=1) as hp:
            e_all = pp.tile([128, 4 * KW], f16)       # gate planes, d-major
            h_a = hp.tile([128, KW], f16)
            h_b = hp.tile([128, KW], f16)
            w2c = pp.tile([MID, KD], f16)
            bmid = pp.tile([MID, 1], f32)
            b2c = pp.tile([KD, 1], f32)
            s_up = pp.tile([128, 128], f16)           # out[p] = v[p-1]
            s_dn = pp.tile([128, 128], f16)           # out[p] = v[p+1]
            iden = pp.tile([128, 128], f16)
            osum = pp.tile([KD, KD], f16)

            # ================= phase A: guidance =================
            with tc.tile_pool(name="w1p", bufs=1) as w1p:
                w1 = w1p.tile([128, 2, 9, MID], f16)

                with tc.tile_pool(name="frows", bufs=4) as frp, \
                     tc.tile_pool(name="xrow", bufs=3) as xrp, \
                     tc.tile_pool(name="estrip", bufs=6) as esp, \
                     tc.tile_pool(name="recip", bufs=3) as recp, \
                     tc.tile_pool(name="psA", bufs=4, space="PSUM") as psA, \
                     tc.tile_pool(name="psG", bufs=2, space="PSUM") as psG, \
                     tc.tile_pool(name="psS", bufs=2, space="PSUM") as psS:
                    n_groups = H // RG
                    ftiles = {}

                    def load_group(g):
                        ft = frp.tile([128, 2, RG, W], f16, name=f"ft{g}",
                                      tag="ft")
                        for c in range(2):
                            nc.sync.dma_start(
                                out=ft[:, c],
                                in_=feats_d[c * 128:(c + 1) * 128,
                                            g * RG:(g + 1) * RG, :])
                        ftiles[g] = ft

                    emitted = 0

                    def ensure_groups(upto):
                        nonlocal emitted
                        while emitted < min(upto, n_groups):
                            load_group(emitted)
                            emitted += 1

                    # startup order: chunk-0 weights + first feats groups
                    # first; constants (not needed until later) after.
                    nc.sync.dma_start(out=w1[:, 0], in_=w1t_d[:, 0])
                    ensure_groups(2)
                    nc.sync.dma_start(out=w1[:, 1], in_=w1t_d[:, 1])
                    nc.sync.dma_start(out=bmid[:], in_=bmid_d[:])
                    nc.sync.dma_start(out=b2c[:], in_=b2_d[:])
                    nc.sync.dma_start(out=w2c[:], in_=w2t_d[:])
                    nc.sync.dma_start(out=osum[:], in_=osum_d[:])
                    nc.sync.dma_start(out=s_up[:], in_=sup_d[:])
                    nc.sync.dma_start(out=s_dn[:], in_=sdn_d[:])
                    nc.sync.dma_start(out=iden[:], in_=idn_d[:])

                    # tap order: full-coverage ky=1 taps first & last so the
                    # start/stop matmuls cover every PSUM element; chunk-0
                    # taps lead so they only need the first w1 DMA.
                    taps = [(0, 1, 1)]
                    for c in range(2):
                        for ky in range(3):
                            for kx in range(3):
                                if (c, ky, kx) not in ((0, 1, 1), (1, 1, 1)):
                                    taps.append((c, ky, kx))
                    taps.append((1, 1, 1))

                    accs, xrs, accgs, ess = {}, {}, {}, {}

                    def emit_taps(wg):
                        for ti, (c, ky, kx) in enumerate(taps):
                            lw = w1[:, c, ky * 3 + kx, :]
                            first = ti == 0
                            last = ti == len(taps) - 1
                            for y in wg:
                                acc = accs[y]
                                rows = [(r, y + r + ky - 1) for r in range(2)
                                        if 0 <= y + r + ky - 1 < H]
                                mms = []
                                if (len(rows) == 2
                                        and rows[0][1] // RG == rows[1][1] // RG):
                                    g, ro = rows[0][1] // RG, rows[0][1] % RG
                                    mms.append((ftiles[g][:, c, ro:ro + 2, :],
                                                acc[:, 0:2, :]))
                                else:
                                    for (r, yin) in rows:
                                        g, ro = yin // RG, yin % RG
                                        mms.append((ftiles[g][:, c, ro, :],
                                                    acc[:, r, :]))
                                for rhs_full, oap in mms:
                                    if kx == 0:
                                        rhs = rhs_full[..., 0:W - 1]
                                        oap = oap[..., 1:W]
                                    elif kx == 2:
                                        rhs = rhs_full[..., 1:W]
                                        oap = oap[..., 0:W - 1]
                                    else:
                                        rhs = rhs_full
                                    nc.tensor.matmul(out=oap, lhsT=lw, rhs=rhs,
                                                     start=first, stop=last)

                    def emit_post_a(wg):
                        # relus first (ACT), then conv1x1s (PE), then exps
                        # (ACT): keeps each engine FIFO free of cross-convoys.
                        for y in wg:
                            xr = xrp.tile([MID, 2, W], f16, name="xr")
                            nc.scalar.activation(xr[:], accs[y][:], Act.Relu,
                                                 bias=bmid[:], scale=1.0)
                            xrs[y] = xr
                        for y in wg:
                            accg = psG.tile([KD, 2, W], f32, name="accg")
                            nc.tensor.matmul(out=accg[:], lhsT=w2c[:],
                                             rhs=xrs[y][:], start=True,
                                             stop=True)
                            accgs[y] = accg
                        for y in wg:
                            es = esp.tile([KD, 2, W], f16, name="es")
                            nc.scalar.activation(es[:], accgs[y][:], Act.Exp,
                                                 bias=b2c[:], scale=1.0)
                            ess[y] = es

                    def emit_post_b(wg):
                        # softmax normalization fused into phase A: direction
                        # sums via tiled-eye matmul, fast reciprocal +
                        # normalize on idle DVE, scatter from idle GpSimd.
                        sps = {}
                        for y in wg:
                            sp = psS.tile([KD, 2, W], f32, name="sum")
                            nc.tensor.matmul(out=sp[:], lhsT=osum[:],
                                             rhs=ess[y][:], start=True,
                                             stop=True)
                            sps[y] = sp
                        for y in wg:
                            rec = recp.tile([KD, 2, W], f32, name="rec")
                            nc.vector.reciprocal_approx_fast(out=rec[:],
                                                             in_=sps[y][:])
                            nc.vector.tensor_tensor(out=ess[y][:],
                                                    in0=ess[y][:], in1=rec[:],
                                                    op=Alu.mult)
                        for y in wg:
                            for r in range(2):
                                nc.gpsimd.dma_start(
                                    out=e_all[y + r:y + r + 1, :].rearrange(
                                        "p (c x) -> p c x", c=KD),
                                    in_=ess[y][:, r, :])

                    pairs = list(range(0, H, 2))
                    wgs = [pairs[i:i + WG] for i in range(0, len(pairs), WG)]
                    for i, wg in enumerate(wgs):
                        ensure_groups((wg[-1] + 2) // RG + 2)
                        for y in wg:
                            accs[y] = psA.tile([MID, 2, W], f32,
                                               name=f"acc{y}", tag="acc")
                        emit_taps(wg)
                        if i > 0:
                            emit_post_a(wgs[i - 1])
                        if i > 1:
                            emit_post_b(wgs[i - 2])
                        if i == 3:
                            # h0 load, placed away from the startup DMA burst
                            nc.sync.dma_start(out=h_a[:],
                                              in_=logits_d[:, :, :])
                    emit_post_a(wgs[-1])
                    emit_post_b(wgs[-2])
                    emit_post_b(wgs[-1])

            # ============ bridge: pre-shift gu/gd planes along y ============
            # gu'[y] = gu[y+1] (lhsT=s_dn) ; gd'[y] = gd[y-1] (lhsT=s_up)
            # high-k chunks first: phase B's first third is k16..19.
            with tc.tile_pool(name="psP", bufs=4, space="PSUM") as psP:
                for d, mat in ((2, s_dn), (3, s_up)):
                    for k0 in reversed(range(0, K, 2)):
                        ck = min(2, K - k0)
                        src = e_all[:, d * KW + k0 * W:d * KW + (k0 + ck) * W]
                        pps = psP.tile([128, 2 * W], f32, name="pps")
                        nc.tensor.matmul(out=pps[:, 0:ck * W], lhsT=mat[:],
                                         rhs=src, start=True, stop=True)
                        nc.scalar.activation(src, pps[:, 0:ck * W], Act.Copy)

            # ================= phase B: recurrence =================
            # small third first, with its q_u/q_d on GpSimd: the slow engine
            # works concurrently with DVE's two big thirds.
            thirds = [(16, 19), (0, 8), (8, 16)]
            with tc.tile_pool(name="qp", bufs=1) as qp, \
                 tc.tile_pool(name="psB", bufs=2, space="PSUM") as psB:
                q_u = qp.tile([128, KW], f16)
                q_d = qp.tile([128, KW], f16)
                q_l = qp.tile([128, KW], f16)
                q_r = qp.tile([128, KW], f16)
                cur, nxt = h_a, h_b
                for t in range(T_STEPS):
                    for (k0, k1) in thirds:
                        nk = k1 - k0
                        f0, f1 = k0 * W, k1 * W
                        hseg = cur[:, f0:f1]
                        eu = nc.gpsimd if nk < 8 else nc.vector
                        # gate products; gl/gr consumed via +-1 flat views
                        eu.tensor_tensor(
                            out=q_u[:, f0:f1], in0=e_all[:, 2 * KW + f0:
                                                         2 * KW + f1],
                            in1=hseg, op=Alu.mult)
                        eu.tensor_tensor(
                            out=q_d[:, f0:f1], in0=e_all[:, 3 * KW + f0:
                                                         3 * KW + f1],
                            in1=hseg, op=Alu.mult)
                        nc.vector.tensor_tensor(
                            out=q_l[:, f0:f1], in0=e_all[:, f0 + 1:f1 + 1],
                            in1=hseg, op=Alu.mult)
                        nc.vector.tensor_tensor(
                            out=q_r[:, f0:f1], in0=e_all[:, KW + f0 - 1:
                                                         KW + f1 - 1],
                            in1=hseg, op=Alu.mult)
                        ps = psB.tile([128, 8, W], f32, name="ps")
                        chunks = [(a, min(a + 2, nk)) for a in range(0, nk, 2)]
                        for (a, b) in chunks:
                            nc.tensor.matmul(
                                out=ps[:, a:b, :], lhsT=s_up[:],
                                rhs=q_u[:, f0 + a * W:f0 + b * W],
                                start=True, stop=False)
                        for (a, b) in chunks:
                            nc.tensor.matmul(
                                out=ps[:, a:b, :], lhsT=s_dn[:],
                                rhs=q_d[:, f0 + a * W:f0 + b * W],
                                start=False, stop=False)
                        for (a, b) in chunks:
                            # agg[x] += q_l[x-1] for x>=1
                            nc.tensor.matmul(
                                out=ps[:, a:b, 1:W], lhsT=iden[:],
                                rhs=q_l[:, f0 + a * W:f0 + b * W].rearrange(
                                    "p (k x) -> p k x", k=b - a)[:, :, 0:W - 1],
                                start=False, stop=False)
                        for (a, b) in chunks:
                            # agg[x] += q_r[x+1] for x<W-1
                            nc.tensor.matmul(
                                out=ps[:, a:b, 0:W - 1], lhsT=iden[:],
                                rhs=q_r[:, f0 + a * W:f0 + b * W].rearrange(
                                    "p (k x) -> p k x", k=b - a)[:, :, 1:W],
                                start=False, stop=True)
                        nc.scalar.activation(nxt[:, f0:f1], ps[:, 0:nk, :],
                                             Act.Copy)
                    cur, nxt = nxt, cur

                nc.sync.dma_start(out=out_d[:, :, :],
                                  in_=cur[:].rearrange("p (k x) -> p k x", k=K))

    nc.compile()
    return nc


_NC_CACHE = None


def kernel(feats, logits, w1, gamma, beta, mean, var, w2, b2):
    global _NC_CACHE
    from concourse.bass_utils import run_bass_kernel_spmd

    feats = np.asarray(feats, dtype=np.float32)
    logits = np.asarray(logits, dtype=np.float32)
    w1 = np.asarray(w1, dtype=np.float32)
    w2 = np.asarray(w2, dtype=np.float32)
    b2 = np.asarray(b2, dtype=np.float32)
    gamma = np.asarray(gamma, dtype=np.float32)
    beta = np.asarray(beta, dtype=np.float32)
    mean = np.asarray(mean, dtype=np.float32)
    var = np.asarray(var, dtype=np.float32)

    inv = gamma / np.sqrt(var + EPS)
    w1f = (w1 * inv[:, None, None, None]).astype(np.float32)  # [MID,CIN,3,3]
    bmid = (beta - mean * inv).astype(np.float32)[:, None]    # [MID,1]
    # [cin_in_chunk 128, chunk 2, tap 9, mid 128]
    w1t = (w1f.transpose(1, 2, 3, 0)                  # [CIN,3,3,MID]
           .reshape(2, 128, 9, MID)
           .transpose(1, 0, 2, 3)).astype(np.float16).copy()
    # d-major output channel order: new channel p = d*K + k <- old k*4 + d
    perm = np.array([k * 4 + d for d in range(4) for k in range(K)])
    w2t = w2.reshape(KD, MID)[perm].T.astype(np.float16).copy()  # [MID,KD]
    b2c = b2[perm][:, None].astype(np.float32).copy()
    s_up = np.eye(128, k=1, dtype=np.float16)         # out[m]=v[m-1]
    s_dn = np.eye(128, k=-1, dtype=np.float16)        # out[m]=v[m+1]
    idn = np.eye(128, dtype=np.float16)
    osum = np.tile(np.eye(K, dtype=np.float16), (4, 4))  # [KD,KD] dir-sum

    if _NC_CACHE is None:
        _NC_CACHE = _build()
    nc = _NC_CACHE

    in_maps = []
    for i in range(B):
        in_maps.append({
            "feats": np.ascontiguousarray(feats[i]).astype(np.float16),
            "logits": np.ascontiguousarray(
                logits[i].transpose(1, 0, 2)).astype(np.float16),
            "w1t": w1t, "bmid": bmid, "w2t": w2t, "b2": b2c,
            "sup": s_up, "sdn": s_dn, "idn": idn, "osum": osum,
        })

    trace = bool(os.environ.get("KTRACE"))
    res = run_bass_kernel_spmd(nc, in_maps, list(range(B)), trace=trace)
    if trace and res.exec_time_ns is not None:
        print(f"HW exec time: {res.exec_time_ns} ns")
    out = np.stack([res.results[i]["out"] for i in range(B)], axis=0)
    return out.transpose(0, 2, 1, 3).astype(np.float32)


if __name__ == "__main__":
    rng = np.random.default_rng(0)
    ins = {
        "feats": rng.standard_normal((B, CIN, H, W), dtype=np.float32),
        "logits": rng.standard_normal((B, K, H, W), dtype=np.float32),
        "w1": rng.standard_normal((MID, CIN, 3, 3), dtype=np.float32) / 48.0,
        "gamma": rng.standard_normal(MID).astype(np.float32) * 0.1 + 1.0,
        "beta": rng.standard_normal(MID).astype(np.float32) * 0.1,
        "mean": rng.standard_normal(MID).astype(np.float32) * 0.1,
        "var": rng.random(MID).astype(np.float32) + 0.5,
        "w2": rng.standard_normal((KD, MID, 1, 1)).astype(np.float32) / 11.3,
        "b2": rng.standard_normal(KD).astype(np.float32) * 0.01,
    }
    o = kernel(**ins)
    print("kernel out", o.shape, o.dtype, np.abs(o).mean())


# revision 19
# speedup vs baseline: 1.9834x; 1.0232x over previous
"""MCSPN Trainium2 kernel: guidance convs + fused softmax gates + 4-step CSPN.

Data-parallel over batch: 8 images -> 8 NeuronCores, one image per core.
fp16 everywhere (PSUM accum stays f32).

Per core:
  phase A: conv3x3 as 18 accumulating fp16 matmuls per row-pair, weight-cycled
           over groups of 2 row-pairs; feats tiles UNGUARDED/contiguous (4KB
           DMA packets); horizontal taps use edge-trimmed windows. Pipelined
           post-stages (one/two blocks behind the taps so no engine convoys):
           relu+BN (ACT) -> conv1x1 d-major (PE) -> exp (ACT) ->
           softmax fused in: partition-sum over the 4 directions via a
           tiled-eye matmul (PE) -> fast reciprocal + in-place normalize on
           the otherwise-idle DVE -> gate scatter DMAs issued from the idle
           GpSimd queue (keeps Sync free for feats loads).
  bridge:  pre-shift gu/gd gate planes along y (PE shift-matmul + ACT copy).
  phase B: packed h [128, K*W]; per step: 4 gate-product mults (DVE; GpSimd
           assists on the small k-chunk), 4 shift matmuls per 2-k chunk
           accumulating into PSUM (up/down via sub/super-diagonal, left/right
           via identity over shifted q windows), ACT evacuates PSUM -> nxt.
"""
import os
import sys

sys.path.insert(0, "/opt/trn_rl_repo")

import numpy as np

B, CIN, H, W = 8, 256, 128, 256
K = 19
MID = 128
KD = 4 * K  # 76
EPS = 1e-5
T_STEPS = 4
KW = K * W  # 4864
RG = 8      # feats rows per DMA group
WG = 2      # row-pairs per weight-cycle group


def _build():
    import concourse.bacc as bacc
    import concourse.mybir as mybir
    import concourse.tile as tile
    from concourse import bass

    f32 = mybir.dt.float32
    f16 = mybir.dt.float16
    Act = mybir.ActivationFunctionType
    Alu = mybir.AluOpType

    nc = bacc.Bacc("TRN2", target_bir_lowering=False)

    feats_d = nc.dram_tensor("feats", [CIN, H, W], f16, kind="ExternalInput")
    logits_d = nc.dram_tensor("logits", [H, K, W], f16, kind="ExternalInput")
    w1t_d = nc.dram_tensor("w1t", [128, 2, 9, MID], f16, kind="ExternalInput")
    bmid_d = nc.dram_tensor("bmid", [MID, 1], f32, kind="ExternalInput")
    w2t_d = nc.dram_tensor("w2t", [MID, KD], f16, kind="ExternalInput")
    b2_d = nc.dram_tensor("b2", [KD, 1], f32, kind="ExternalInput")
    sup_d = nc.dram_tensor("sup", [128, 128], f16, kind="ExternalInput")
    sdn_d = nc.dram_tensor("sdn", [128, 128], f16, kind="ExternalInput")
    idn_d = nc.dram_tensor("idn", [128, 128], f16, kind="ExternalInput")
    osum_d = nc.dram_tensor("osum", [KD, KD], f16, kind="ExternalInput")
    zer_d = nc.dram_tensor("zer", [1, KW], f16, kind="ExternalInput")
    out_d = nc.dram_tensor("out", [H, K, W], f16, kind="ExternalOutput")

    with tile.TileContext(nc) as tc:
        with tc.tile_pool(name="persist", bufs=1) as pp, \
             tc.tile_pool(name="hpool", bufs=1) as hp:
            e_all = pp.tile([128, 4 * KW], f16)       # gate planes, d-major
            h_a = hp.tile([128, KW], f16)
            h_b = hp.tile([128, KW], f16)
            w2c = pp.tile([MID, KD], f16)
            bmid = pp.tile([MID, 1], f32)
            b2c = pp.tile([KD, 1], f32)
            s_up = pp.tile([128, 128], f16)           # out[p] = v[p-1]
            s_dn = pp.tile([128, 128], f16)           # out[p] = v[p+1]
            iden = pp.tile([128, 128], f16)
            osum = pp.tile([KD, KD], f16)

            # ================= phase A: guidance =================
            with tc.tile_pool(name="w1p", bufs=1) as w1p:
                w1 = w1p.tile([128, 2, 9, MID], f16)

                with tc.tile_pool(name="frows", bufs=4) as frp, \
                     tc.tile_pool(name="xrow", bufs=4) as xrp, \
                     tc.tile_pool(name="estrip", bufs=6) as esp, \
                     tc.tile_pool(name="recip", bufs=3) as recp, \
                     tc.tile_pool(name="psA", bufs=4, space="PSUM") as psA, \
                     tc.tile_pool(name="psG", bufs=2, space="PSUM") as psG, \
                     tc.tile_pool(name="psS", bufs=2, space="PSUM") as psS:
                    n_groups = H // RG
                    ftiles = {}

                    def load_group(g):
                        ft = frp.tile([128, 2, RG, W], f16, name=f"ft{g}",
                                      tag="ft")
                        for c in range(2):
                            nc.sync.dma_start(
                                out=ft[:, c],
                                in_=feats_d[c * 128:(c + 1) * 128,
                                            g * RG:(g + 1) * RG, :])
                        ftiles[g] = ft

                    emitted = 0

                    def ensure_groups(upto):
                        nonlocal emitted
                        while emitted < min(upto, n_groups):
                            load_group(emitted)
                            emitted += 1

                    # startup order: chunk-0 weights + first feats groups
                    # first; constants (not needed until later) after.
                    nc.sync.dma_start(out=w1[:, 0], in_=w1t_d[:, 0])
                    ensure_groups(2)
                    nc.sync.dma_start(out=w1[:, 1], in_=w1t_d[:, 1])
                    nc.sync.dma_start(out=bmid[:], in_=bmid_d[:])
                    nc.sync.dma_start(out=b2c[:], in_=b2_d[:])
                    nc.sync.dma_start(out=w2c[:], in_=w2t_d[:])
                    nc.sync.dma_start(out=osum[:], in_=osum_d[:])
                    nc.sync.dma_start(out=s_up[:], in_=sup_d[:])
                    nc.sync.dma_start(out=s_dn[:], in_=sdn_d[:])
                    nc.sync.dma_start(out=iden[:], in_=idn_d[:])

                    # tap order: full-coverage ky=1 taps first & last so the
                    # start/stop matmuls cover every PSUM element; chunk-0
                    # taps lead so they only need the first w1 DMA.
                    taps = [(0, 1, 1)]
                    for c in range(2):
                        for ky in range(3):
                            for kx in range(3):
                                if (c, ky, kx) not in ((0, 1, 1), (1, 1, 1)):
                                    taps.append((c, ky, kx))
                    taps.append((1, 1, 1))

                    accs, xrs, accgs, ess = {}, {}, {}, {}

                    def emit_taps(wg):
                        for ti, (c, ky, kx) in enumerate(taps):
                            lw = w1[:, c, ky * 3 + kx, :]
                            first = ti == 0
                            last = ti == len(taps) - 1
                            for y in wg:
                                acc = accs[y]
                                rows = [(r, y + r + ky - 1) for r in range(2)
                                        if 0 <= y + r + ky - 1 < H]
                                mms = []
                                if (len(rows) == 2
                                        and rows[0][1] // RG == rows[1][1] // RG):
                                    g, ro = rows[0][1] // RG, rows[0][1] % RG
                                    mms.append((ftiles[g][:, c, ro:ro + 2, :],
                                                acc[:, 0:2, :]))
                                else:
                                    for (r, yin) in rows:
                                        g, ro = yin // RG, yin % RG
                                        mms.append((ftiles[g][:, c, ro, :],
                                                    acc[:, r, :]))
                                for rhs_full, oap in mms:
                                    if kx == 0:
                                        rhs = rhs_full[..., 0:W - 1]
                                        oap = oap[..., 1:W]
                                    elif kx == 2:
                                        rhs = rhs_full[..., 1:W]
                                        oap = oap[..., 0:W - 1]
                                    else:
                                        rhs = rhs_full
                                    nc.tensor.matmul(out=oap, lhsT=lw, rhs=rhs,
                                                     start=first, stop=last)

                    # edge rows never written by the shifted scatters: must
                    # be finite (0) or the 0-coeff NaN would poison shift MMs.
                    # (fp16 memset crashes walrus; DMA zeros instead)
                    nc.sync.dma_start(out=e_all[127:128, 2 * KW:3 * KW],
                                      in_=zer_d[:])
                    nc.sync.dma_start(out=e_all[0:1, 3 * KW:4 * KW],
                                      in_=zer_d[:])

                    def emit_relus(wg):
                        for y in wg:
                            xr = xrp.tile([MID, 2, W], f16, name="xr")
                            nc.scalar.activation(xr[:], accs[y][:], Act.Relu,
                                                 bias=bmid[:], scale=1.0)
                            xrs[y] = xr

                    def emit_c1(wg):
                        for y in wg:
                            accg = psG.tile([KD, 2, W], f32, name="accg")
                            nc.tensor.matmul(out=accg[:], lhsT=w2c[:],
                                             rhs=xrs[y][:], start=True,
                                             stop=True)
                            accgs[y] = accg
                        for y in wg:
                            es = esp.tile([KD, 2, W], f16, name="es")
                            nc.scalar.activation(es[:], accgs[y][:], Act.Exp,
                                                 bias=b2c[:], scale=1.0)
                            ess[y] = es

                    def emit_post_b(wg):
                        # softmax normalization fused into phase A: direction
                        # sums via tiled-eye matmul, fast reciprocal +
                        # normalize on idle DVE. Scatters apply the gu/gd
                        # y-pre-shift for free: gu of row y lands at row y-1,
                        # gd at y+1 (edge rows dropped; consumers never read
                        # the dropped positions).
                        sps = {}
                        for y in wg:
                            sp = psS.tile([KD, 2, W], f32, name="sum")
                            nc.tensor.matmul(out=sp[:], lhsT=osum[:],
                                             rhs=ess[y][:], start=True,
                                             stop=True)
                            sps[y] = sp
                        for y in wg:
                            rec = recp.tile([KD, 2, W], f32, name="rec")
                            nc.vector.reciprocal_approx_fast(out=rec[:],
                                                             in_=sps[y][:])
                            nc.vector.tensor_tensor(out=ess[y][:],
                                                    in0=ess[y][:], in1=rec[:],
                                                    op=Alu.mult)
                        for y in wg:
                            for r in range(2):
                                yy = y + r
                                nc.gpsimd.dma_start(
                                    out=e_all[yy:yy + 1, 0:2 * KW].rearrange(
                                        "p (c x) -> p c x", c=2 * K),
                                    in_=ess[y][0:2 * K, r, :])
                                if yy > 0:
                                    nc.gpsimd.dma_start(
                                        out=e_all[yy - 1:yy,
                                                  2 * KW:3 * KW].rearrange(
                                            "p (c x) -> p c x", c=K),
                                        in_=ess[y][2 * K:3 * K, r, :])
                                if yy < H - 1:
                                    nc.sync.dma_start(
                                        out=e_all[yy + 1:yy + 2,
                                                  3 * KW:4 * KW].rearrange(
                                            "p (c x) -> p c x", c=K),
                                        in_=ess[y][3 * K:4 * K, r, :])

                    pairs = list(range(0, H, 2))
                    wgs = [pairs[i:i + WG] for i in range(0, len(pairs), WG)]
                    for i, wg in enumerate(wgs):
                        ensure_groups((wg[-1] + 2) // RG + 2)
                        if i > 0:
                            emit_relus(wgs[i - 1])
                        for y in wg:
                            accs[y] = psA.tile([MID, 2, W], f32,
                                               name=f"acc{y}", tag="acc")
                        emit_taps(wg)
                        if i > 0:
                            emit_c1(wgs[i - 1])
                        if i > 1:
                            emit_post_b(wgs[i - 2])
                        if i == 3:
                            # h0 load, placed away from the startup DMA burst
                            nc.sync.dma_start(out=h_a[:],
                                              in_=logits_d[:, :, :])
                    emit_relus(wgs[-1])
                    emit_c1(wgs[-1])
                    emit_post_b(wgs[-2])
                    emit_post_b(wgs[-1])

            # ================= phase B: recurrence =================
            thirds = [(0, 8), (8, 16), (16, 19)]
            with tc.tile_pool(name="qp", bufs=1) as qp, \
                 tc.tile_pool(name="psB", bufs=2, space="PSUM") as psB:
                q_u = qp.tile([128, KW], f16)
                q_d = qp.tile([128, KW], f16)
                q_l = qp.tile([128, KW], f16)
                q_r = qp.tile([128, KW], f16)
                cur, nxt = h_a, h_b
                for t in range(T_STEPS):
                    for (k0, k1) in thirds:
                        nk = k1 - k0
                        f0, f1 = k0 * W, k1 * W
                        hseg = cur[:, f0:f1]
                        # gate products; gl/gr consumed via +-1 flat views
                        nc.vector.tensor_tensor(
                            out=q_u[:, f0:f1], in0=e_all[:, 2 * KW + f0:
                                                         2 * KW + f1],
                            in1=hseg, op=Alu.mult)
                        nc.vector.tensor_tensor(
                            out=q_d[:, f0:f1], in0=e_all[:, 3 * KW + f0:
                                                         3 * KW + f1],
                            in1=hseg, op=Alu.mult)
                        nc.vector.tensor_tensor(
                            out=q_l[:, f0:f1], in0=e_all[:, f0 + 1:f1 + 1],
                            in1=hseg, op=Alu.mult)
                        nc.vector.tensor_tensor(
                            out=q_r[:, f0:f1], in0=e_all[:, KW + f0 - 1:
                                                         KW + f1 - 1],
                            in1=hseg, op=Alu.mult)
                        ps = psB.tile([128, 8, W], f32, name="ps")
                        chunks = [(a, min(a + 2, nk)) for a in range(0, nk, 2)]
                        for (a, b) in chunks:
                            nc.tensor.matmul(
                                out=ps[:, a:b, :], lhsT=s_up[:],
                                rhs=q_u[:, f0 + a * W:f0 + b * W],
                                start=True, stop=False)
                        for (a, b) in chunks:
                            nc.tensor.matmul(
                                out=ps[:, a:b, :], lhsT=s_dn[:],
                                rhs=q_d[:, f0 + a * W:f0 + b * W],
                                start=False, stop=False)
                        for (a, b) in chunks:
                            # agg[x] += q_l[x-1] for x>=1
                            nc.tensor.matmul(
                                out=ps[:, a:b, 1:W], lhsT=iden[:],
                                rhs=q_l[:, f0 + a * W:f0 + b * W].rearrange(
                                    "p (k x) -> p k x", k=b - a)[:, :, 0:W - 1],
                                start=False, stop=False)
                        for (a, b) in chunks:
                            # agg[x] += q_r[x+1] for x<W-1
                            nc.tensor.matmul(
                                out=ps[:, a:b, 0:W - 1], lhsT=iden[:],
                                rhs=q_r[:, f0 + a * W:f0 + b * W].rearrange(
                                    "p (k x) -> p k x", k=b - a)[:, :, 1:W],
                                start=False, stop=True)
                        nc.scalar.activation(nxt[:, f0:f1], ps[:, 0:nk, :],
                                             Act.Copy)
                    cur, nxt = nxt, cur

                nc.sync.dma_start(out=out_d[:, :, :],
                                  in_=cur[:].rearrange("p (k x) -> p k x", k=K))

    nc.compile()
    return nc


_NC_CACHE = None


def kernel(feats, logits, w1, gamma, beta, mean, var, w2, b2):
    global _NC_CACHE
    from concourse.bass_utils import run_bass_kernel_spmd

    feats = np.asarray(feats, dtype=np.float32)
    logits = np.asarray(logits, dtype=np.float32)
    w1 = np.asarray(w1, dtype=np.float32)
    w2 = np.asarray(w2, dtype=np.float32)
    b2 = np.asarray(b2, dtype=np.float32)
    gamma = np.asarray(gamma, dtype=np.float32)
    beta = np.asarray(beta, dtype=np.float32)
    mean = np.asarray(mean, dtype=np.float32)
    var = np.asarray(var, dtype=np.float32)

    inv = gamma / np.sqrt(var + EPS)
    w1f = (w1 * inv[:, None, None, None]).astype(np.float32)  # [MID,CIN,3,3]
    bmid = (beta - mean * inv).astype(np.float32)[:, None]    # [MID,1]
    # [cin_in_chunk 128, chunk 2, tap 9, mid 128]
    w1t = (w1f.transpose(1, 2, 3, 0)                  # [CIN,3,3,MID]
           .reshape(2, 128, 9, MID)
           .transpose(1, 0, 2, 3)).astype(np.float16).copy()
    # d-major output channel order: new channel p = d*K + k <- old k*4 + d
    perm = np.array([k * 4 + d for d in range(4) for k in range(K)])
    w2t = w2.reshape(KD, MID)[perm].T.astype(np.float16).copy()  # [MID,KD]
    b2c = b2[perm][:, None].astype(np.float32).copy()
    s_up = np.eye(128, k=1, dtype=np.float16)         # out[m]=v[m-1]
    s_dn = np.eye(128, k=-1, dtype=np.float16)        # out[m]=v[m+1]
    idn = np.eye(128, dtype=np.float16)
    osum = np.tile(np.eye(K, dtype=np.float16), (4, 4))  # [KD,KD] dir-sum

    if _NC_CACHE is None:
        _NC_CACHE = _build()
    nc = _NC_CACHE

    in_maps = []
    for i in range(B):
        in_maps.append({
            "feats": np.ascontiguousarray(feats[i]).astype(np.float16),
            "logits": np.ascontiguousarray(
                logits[i].transpose(1, 0, 2)).astype(np.float16),
            "w1t": w1t, "bmid": bmid, "w2t": w2t, "b2": b2c,
            "sup": s_up, "sdn": s_dn, "idn": idn, "osum": osum,
            "zer": np.zeros((1, KW), np.float16),
        })

    trace = bool(os.environ.get("KTRACE"))
    res = run_bass_kernel_spmd(nc, in_maps, list(range(B)), trace=trace)
    if trace and res.exec_time_ns is not None:
        print(f"HW exec time: {res.exec_time_ns} ns")
    out = np.stack([res.results[i]["out"] for i in range(B)], axis=0)
    return out.transpose(0, 2, 1, 3).astype(np.float32)


if __name__ == "__main__":
    rng = np.random.default_rng(0)
    ins = {
        "feats": rng.standard_normal((B, CIN, H, W), dtype=np.float32),
        "logits": rng.standard_normal((B, K, H, W), dtype=np.float32),
        "w1": rng.standard_normal((MID, CIN, 3, 3), dtype=np.float32) / 48.0,
        "gamma": rng.standard_normal(MID).astype(np.float32) * 0.1 + 1.0,
        "beta": rng.standard_normal(MID).astype(np.float32) * 0.1,
        "mean": rng.standard_normal(MID).astype(np.float32) * 0.1,
        "var": rng.random(MID).astype(np.float32) + 0.5,
        "w2": rng.standard_normal((KD, MID, 1, 1)).astype(np.float32) / 11.3,
        "b2": rng.standard_normal(KD).astype(np.float32) * 0.01,
    }
    o = kernel(**ins)
    print("kernel out", o.shape, o.dtype, np.abs(o).mean())


# revision 20
# speedup vs baseline: 1.9859x; 1.0012x over previous
"""MCSPN Trainium2 kernel: guidance convs + fused softmax gates + 4-step CSPN.

Data-parallel over batch: 8 images -> 8 NeuronCores, one image per core.
fp16 everywhere (PSUM accum stays f32).

Per core:
  phase A: conv3x3 as 18 accumulating fp16 matmuls per row-pair, weight-cycled
           over groups of 2 row-pairs; feats tiles UNGUARDED/contiguous (4KB
           DMA packets); horizontal taps use edge-trimmed windows. Pipelined
           post-stages (one/two blocks behind the taps so no engine convoys):
           relu+BN (ACT) -> conv1x1 d-major (PE) -> exp (ACT) ->
           softmax fused in: partition-sum over the 4 directions via a
           tiled-eye matmul (PE) -> fast reciprocal + in-place normalize on
           the otherwise-idle DVE -> gate scatter DMAs issued from the idle
           GpSimd queue (keeps Sync free for feats loads).
  bridge:  pre-shift gu/gd gate planes along y (PE shift-matmul + ACT copy).
  phase B: packed h [128, K*W]; per step: 4 gate-product mults (DVE; GpSimd
           assists on the small k-chunk), 4 shift matmuls per 2-k chunk
           accumulating into PSUM (up/down via sub/super-diagonal, left/right
           via identity over shifted q windows), ACT evacuates PSUM -> nxt.
"""
import os
import sys

sys.path.insert(0, "/opt/trn_rl_repo")

import numpy as np

B, CIN, H, W = 8, 256, 128, 256
K = 19
MID = 128
KD = 4 * K  # 76
EPS = 1e-5
T_STEPS = 4
KW = K * W  # 4864
RG = 8      # feats rows per DMA group
WG = 2      # row-pairs per weight-cycle group


def _build():
    import concourse.bacc as bacc
    import concourse.mybir as mybir
    import concourse.tile as tile
    from concourse import bass

    f32 = mybir.dt.float32
    f16 = mybir.dt.float16
    Act = mybir.ActivationFunctionType
    Alu = mybir.AluOpType

    nc = bacc.Bacc("TRN2", target_bir_lowering=False)

    feats_d = nc.dram_tensor("feats", [CIN, H, W], f16, kind="ExternalInput")
    logits_d = nc.dram_tensor("logits", [H, K, W], f16, kind="ExternalInput")
    w1t_d = nc.dram_tensor("w1t", [128, 2, 9, MID], f16, kind="ExternalInput")
    bmid_d = nc.dram_tensor("bmid", [MID, 1], f32, kind="ExternalInput")
    w2t_d = nc.dram_tensor("w2t", [MID, KD], f16, kind="ExternalInput")
    b2_d = nc.dram_tensor("b2", [KD, 1], f32, kind="ExternalInput")
    sup_d = nc.dram_tensor("sup", [128, 128], f16, kind="ExternalInput")
    sdn_d = nc.dram_tensor("sdn", [128, 128], f16, kind="ExternalInput")
    idn_d = nc.dram_tensor("idn", [128, 128], f16, kind="ExternalInput")
    osum_d = nc.dram_tensor("osum", [KD, KD], f16, kind="ExternalInput")
    zer_d = nc.dram_tensor("zer", [1, KW], f16, kind="ExternalInput")
    out_d = nc.dram_tensor("out", [H, K, W], f16, kind="ExternalOutput")

    with tile.TileContext(nc) as tc:
        with tc.tile_pool(name="persist", bufs=1) as pp, \
             tc.tile_pool(name="hpool", bufs=1) as hp:
            e_all = pp.tile([128, 4 * KW], f16)       # gate planes, d-major
            h_a = hp.tile([128, KW], f16)
            h_b = hp.tile([128, KW], f16)
            w2c = pp.tile([MID, KD], f16)
            bmid = pp.tile([MID, 1], f32)
            b2c = pp.tile([KD, 1], f32)
            s_up = pp.tile([128, 128], f16)           # out[p] = v[p-1]
            s_dn = pp.tile([128, 128], f16)           # out[p] = v[p+1]
            iden = pp.tile([128, 128], f16)
            osum = pp.tile([KD, KD], f16)

            # ================= phase A: guidance =================
            with tc.tile_pool(name="w1p", bufs=1) as w1p:
                w1 = w1p.tile([128, 2, 9, MID], f16)

                with tc.tile_pool(name="frows", bufs=4) as frp, \
                     tc.tile_pool(name="xrow", bufs=4) as xrp, \
                     tc.tile_pool(name="estrip", bufs=6) as esp, \
                     tc.tile_pool(name="recip", bufs=3) as recp, \
                     tc.tile_pool(name="psA", bufs=4, space="PSUM") as psA, \
                     tc.tile_pool(name="psG", bufs=2, space="PSUM") as psG, \
                     tc.tile_pool(name="psS", bufs=2, space="PSUM") as psS:
                    n_groups = H // RG
                    ftiles = {}

                    def load_group(g):
                        ft = frp.tile([128, 2, RG, W], f16, name=f"ft{g}",
                                      tag="ft")
                        for c in range(2):
                            nc.sync.dma_start(
                                out=ft[:, c],
                                in_=feats_d[c * 128:(c + 1) * 128,
                                            g * RG:(g + 1) * RG, :])
                        ftiles[g] = ft

                    emitted = 0

                    def ensure_groups(upto):
                        nonlocal emitted
                        while emitted < min(upto, n_groups):
                            load_group(emitted)
                            emitted += 1

                    # startup order: chunk-0 weights + first feats groups
                    # first; constants (not needed until later) after.
                    nc.sync.dma_start(out=w1[:, 0], in_=w1t_d[:, 0])
                    ensure_groups(2)
                    nc.sync.dma_start(out=w1[:, 1], in_=w1t_d[:, 1])
                    nc.sync.dma_start(out=bmid[:], in_=bmid_d[:])
                    nc.sync.dma_start(out=b2c[:], in_=b2_d[:])
                    nc.sync.dma_start(out=w2c[:], in_=w2t_d[:])
                    nc.sync.dma_start(out=osum[:], in_=osum_d[:])
                    nc.sync.dma_start(out=s_up[:], in_=sup_d[:])
                    nc.sync.dma_start(out=s_dn[:], in_=sdn_d[:])
                    nc.sync.dma_start(out=iden[:], in_=idn_d[:])

                    # tap order: full-coverage ky=1 taps first & last so the
                    # start/stop matmuls cover every PSUM element; chunk-0
                    # taps lead so they only need the first w1 DMA.
                    taps = [(0, 1, 1)]
                    for c in range(2):
                        for ky in range(3):
                            for kx in range(3):
                                if (c, ky, kx) not in ((0, 1, 1), (1, 1, 1)):
                                    taps.append((c, ky, kx))
                    taps.append((1, 1, 1))

                    accs, xrs, accgs, ess = {}, {}, {}, {}

                    def emit_taps(wg):
                        for ti, (c, ky, kx) in enumerate(taps):
                            lw = w1[:, c, ky * 3 + kx, :]
                            first = ti == 0
                            last = ti == len(taps) - 1
                            for y in wg:
                                acc = accs[y]
                                rows = [(r, y + r + ky - 1) for r in range(2)
                                        if 0 <= y + r + ky - 1 < H]
                                mms = []
                                if (len(rows) == 2
                                        and rows[0][1] // RG == rows[1][1] // RG):
                                    g, ro = rows[0][1] // RG, rows[0][1] % RG
                                    mms.append((ftiles[g][:, c, ro:ro + 2, :],
                                                acc[:, 0:2, :]))
                                else:
                                    for (r, yin) in rows:
                                        g, ro = yin // RG, yin % RG
                                        mms.append((ftiles[g][:, c, ro, :],
                                                    acc[:, r, :]))
                                for rhs_full, oap in mms:
                                    if kx == 0:
                                        rhs = rhs_full[..., 0:W - 1]
                                        oap = oap[..., 1:W]
                                    elif kx == 2:
                                        rhs = rhs_full[..., 1:W]
                                        oap = oap[..., 0:W - 1]
                                    else:
                                        rhs = rhs_full
                                    nc.tensor.matmul(out=oap, lhsT=lw, rhs=rhs,
                                                     start=first, stop=last)

                    # edge rows never written by the shifted scatters: must
                    # be finite (0) or the 0-coeff NaN would poison shift MMs.
                    # (fp16 memset crashes walrus; DMA zeros instead)
                    nc.sync.dma_start(out=e_all[127:128, 2 * KW:3 * KW],
                                      in_=zer_d[:])
                    nc.sync.dma_start(out=e_all[0:1, 3 * KW:4 * KW],
                                      in_=zer_d[:])

                    def emit_relus(wg):
                        for y in wg:
                            xr = xrp.tile([MID, 2, W], f16, name="xr")
                            nc.scalar.activation(xr[:], accs[y][:], Act.Relu,
                                                 bias=bmid[:], scale=1.0)
                            xrs[y] = xr

                    def emit_c1(wg):
                        for y in wg:
                            accg = psG.tile([KD, 2, W], f32, name="accg")
                            nc.tensor.matmul(out=accg[:], lhsT=w2c[:],
                                             rhs=xrs[y][:], start=True,
                                             stop=True)
                            accgs[y] = accg
                        for y in wg:
                            es = esp.tile([KD, 2, W], f16, name="es")
                            nc.scalar.activation(es[:], accgs[y][:], Act.Exp,
                                                 bias=b2c[:], scale=1.0)
                            ess[y] = es

                    def emit_post_b(wg):
                        # softmax normalization fused into phase A: direction
                        # sums via tiled-eye matmul, fast reciprocal +
                        # normalize on idle DVE. Scatters apply the gu/gd
                        # y-pre-shift for free: gu of row y lands at row y-1,
                        # gd at y+1 (edge rows dropped; consumers never read
                        # the dropped positions).
                        sps = {}
                        for y in wg:
                            sp = psS.tile([KD, 2, W], f32, name="sum")
                            nc.tensor.matmul(out=sp[:], lhsT=osum[:],
                                             rhs=ess[y][:], start=True,
                                             stop=True)
                            sps[y] = sp
                        for y in wg:
                            rec = recp.tile([KD, 2, W], f32, name="rec")
                            nc.vector.reciprocal_approx_fast(out=rec[:],
                                                             in_=sps[y][:])
                            nc.vector.tensor_tensor(out=ess[y][:],
                                                    in0=ess[y][:], in1=rec[:],
                                                    op=Alu.mult)
                        for y in wg:
                            for r in range(2):
                                yy = y + r
                                nc.gpsimd.dma_start(
                                    out=e_all[yy:yy + 1, 0:2 * KW].rearrange(
                                        "p (c x) -> p c x", c=2 * K),
                                    in_=ess[y][0:2 * K, r, :])
                                if yy > 0:
                                    nc.gpsimd.dma_start(
                                        out=e_all[yy - 1:yy,
                                                  2 * KW:3 * KW].rearrange(
                                            "p (c x) -> p c x", c=K),
                                        in_=ess[y][2 * K:3 * K, r, :])
                                if yy < H - 1:
                                    nc.sync.dma_start(
                                        out=e_all[yy + 1:yy + 2,
                                                  3 * KW:4 * KW].rearrange(
                                            "p (c x) -> p c x", c=K),
                                        in_=ess[y][3 * K:4 * K, r, :])

                    # 2-block-lagged post pipeline: all ACT work emitted for a
                    # block (exp of wg i-2, relu of wg i-1) depends only on PE
                    # work from EARLIER blocks, so ACT runs bunched at block
                    # start and never builds a backlog that delays the psA
                    # WAR release for the next tap block.
                    pairs = list(range(0, H, 2))
                    wgs = [pairs[i:i + WG] for i in range(0, len(pairs), WG)]
                    for i, wg in enumerate(wgs):
                        if i > 1:
                            emit_c1(wgs[i - 2])
                        if i > 0:
                            emit_relus(wgs[i - 1])
                        ensure_groups((wg[-1] + 2) // RG + 2)
                        for y in wg:
                            accs[y] = psA.tile([MID, 2, W], f32,
                                               name=f"acc{y}", tag="acc")
                        emit_taps(wg)
                        if i > 2:
                            emit_post_b(wgs[i - 3])
                        if i == 3:
                            # h0 load, placed away from the startup DMA burst
                            nc.sync.dma_start(out=h_a[:],
                                              in_=logits_d[:, :, :])
                    emit_c1(wgs[-2])
                    emit_relus(wgs[-1])
                    emit_post_b(wgs[-3])
                    emit_c1(wgs[-1])
                    emit_post_b(wgs[-2])
                    emit_post_b(wgs[-1])

            # ================= phase B: recurrence =================
            thirds = [(0, 8), (8, 16), (16, 19)]
            with tc.tile_pool(name="qp", bufs=1) as qp, \
                 tc.tile_pool(name="psB", bufs=2, space="PSUM") as psB:
                q_u = qp.tile([128, KW], f16)
                q_d = qp.tile([128, KW], f16)
                q_l = qp.tile([128, KW], f16)
                q_r = qp.tile([128, KW], f16)
                cur, nxt = h_a, h_b
                for t in range(T_STEPS):
                    for (k0, k1) in thirds:
                        nk = k1 - k0
                        f0, f1 = k0 * W, k1 * W
                        hseg = cur[:, f0:f1]
                        # gate products; gl/gr consumed via +-1 flat views
                        nc.vector.tensor_tensor(
                            out=q_u[:, f0:f1], in0=e_all[:, 2 * KW + f0:
                                                         2 * KW + f1],
                            in1=hseg, op=Alu.mult)
                        nc.vector.tensor_tensor(
                            out=q_d[:, f0:f1], in0=e_all[:, 3 * KW + f0:
                                                         3 * KW + f1],
                            in1=hseg, op=Alu.mult)
                        nc.vector.tensor_tensor(
                            out=q_l[:, f0:f1], in0=e_all[:, f0 + 1:f1 + 1],
                            in1=hseg, op=Alu.mult)
                        nc.vector.tensor_tensor(
                            out=q_r[:, f0:f1], in0=e_all[:, KW + f0 - 1:
                                                         KW + f1 - 1],
                            in1=hseg, op=Alu.mult)
                        ps = psB.tile([128, 8, W], f32, name="ps")
                        chunks = [(a, min(a + 2, nk)) for a in range(0, nk, 2)]
                        for (a, b) in chunks:
                            nc.tensor.matmul(
                                out=ps[:, a:b, :], lhsT=s_up[:],
                                rhs=q_u[:, f0 + a * W:f0 + b * W],
                                start=True, stop=False)
                        for (a, b) in chunks:
                            nc.tensor.matmul(
                                out=ps[:, a:b, :], lhsT=s_dn[:],
                                rhs=q_d[:, f0 + a * W:f0 + b * W],
                                start=False, stop=False)
                        for (a, b) in chunks:
                            # agg[x] += q_l[x-1] for x>=1
                            nc.tensor.matmul(
                                out=ps[:, a:b, 1:W], lhsT=iden[:],
                                rhs=q_l[:, f0 + a * W:f0 + b * W].rearrange(
                                    "p (k x) -> p k x", k=b - a)[:, :, 0:W - 1],
                                start=False, stop=False)
                        for (a, b) in chunks:
                            # agg[x] += q_r[x+1] for x<W-1
                            nc.tensor.matmul(
                                out=ps[:, a:b, 0:W - 1], lhsT=iden[:],
                                rhs=q_r[:, f0 + a * W:f0 + b * W].rearrange(
                                    "p (k x) -> p k x", k=b - a)[:, :, 1:W],
                                start=False, stop=True)
                        nc.scalar.activation(nxt[:, f0:f1], ps[:, 0:nk, :],
                                             Act.Copy)
                    cur, nxt = nxt, cur

                nc.sync.dma_start(out=out_d[:, :, :],
                                  in_=cur[:].rearrange("p (k x) -> p k x", k=K))

    nc.compile()
    return nc


_NC_CACHE = None


def kernel(feats, logits, w1, gamma, beta, mean, var, w2, b2):
    global _NC_CACHE
    from concourse.bass_utils import run_bass_kernel_spmd

    feats = np.asarray(feats, dtype=np.float32)
    logits = np.asarray(logits, dtype=np.float32)
    w1 = np.asarray(w1, dtype=np.float32)
    w2 = np.asarray(w2, dtype=np.float32)
    b2 = np.asarray(b2, dtype=np.float32)
    gamma = np.asarray(gamma, dtype=np.float32)
    beta = np.asarray(beta, dtype=np.float32)
    mean = np.asarray(mean, dtype=np.float32)
    var = np.asarray(var, dtype=np.float32)

    inv = gamma / np.sqrt(var + EPS)
    w1f = (w1 * inv[:, None, None, None]).astype(np.float32)  # [MID,CIN,3,3]
    bmid = (beta - mean * inv).astype(np.float32)[:, None]    # [MID,1]
    # [cin_in_chunk 128, chunk 2, tap 9, mid 128]
    w1t = (w1f.transpose(1, 2, 3, 0)                  # [CIN,3,3,MID]
           .reshape(2, 128, 9, MID)
           .transpose(1, 0, 2, 3)).astype(np.float16).copy()
    # d-major output channel order: new channel p = d*K + k <- old k*4 + d
    perm = np.array([k * 4 + d for d in range(4) for k in range(K)])
    w2t = w2.reshape(KD, MID)[perm].T.astype(np.float16).copy()  # [MID,KD]
    b2c = b2[perm][:, None].astype(np.float32).copy()
    s_up = np.eye(128, k=1, dtype=np.float16)         # out[m]=v[m-1]
    s_dn = np.eye(128, k=-1, dtype=np.float16)        # out[m]=v[m+1]
    idn = np.eye(128, dtype=np.float16)
    osum = np.tile(np.eye(K, dtype=np.float16), (4, 4))  # [KD,KD] dir-sum

    if _NC_CACHE is None:
        _NC_CACHE = _build()
    nc = _NC_CACHE

    in_maps = []
    for i in range(B):
        in_maps.append({
            "feats": np.ascontiguousarray(feats[i]).astype(np.float16),
            "logits": np.ascontiguousarray(
                logits[i].transpose(1, 0, 2)).astype(np.float16),
            "w1t": w1t, "bmid": bmid, "w2t": w2t, "b2": b2c,
            "sup": s_up, "sdn": s_dn, "idn": idn, "osum": osum,
            "zer": np.zeros((1, KW), np.float16),
        })

    trace = bool(os.environ.get("KTRACE"))
    res = run_bass_kernel_spmd(nc, in_maps, list(range(B)), trace=trace)
    if trace and res.exec_time_ns is not None:
        print(f"HW exec time: {res.exec_time_ns} ns")
    out = np.stack([res.results[i]["out"] for i in range(B)], axis=0)
    return out.transpose(0, 2, 1, 3).astype(np.float32)


if __name__ == "__main__":
    rng = np.random.default_rng(0)
    ins = {
        "feats": rng.standard_normal((B, CIN, H, W), dtype=np.float32),
        "logits": rng.standard_normal((B, K, H, W), dtype=np.float32),
        "w1": rng.standard_normal((MID, CIN, 3, 3), dtype=np.float32) / 48.0,
        "gamma": rng.standard_normal(MID).astype(np.float32) * 0.1 + 1.0,
        "beta": rng.standard_normal(MID).astype(np.float32) * 0.1,
        "mean": rng.standard_normal(MID).astype(np.float32) * 0.1,
        "var": rng.random(MID).astype(np.float32) + 0.5,
        "w2": rng.standard_normal((KD, MID, 1, 1)).astype(np.float32) / 11.3,
        "b2": rng.standard_normal(KD).astype(np.float32) * 0.01,
    }
    o = kernel(**ins)
    print("kernel out", o.shape, o.dtype, np.abs(o).mean())
